# revision 2
# baseline (speedup 1.0000x reference)
"""Trainium2 Bass kernel for nn_Attention_4612794875918.

Full inputs in, full outputs out. Internally shards across 8 NeuronCores:
core c handles batch b = c//2, head group hg = c%2 (4 heads each) — scores,
y_standard, memory matrix M and the delta scan are independent per (B, nh).

Device does all O(T^2) / O(T*N*D) matmul work per (b,h):
  - scoresT = rope(Q) @ rope(Q)^T (strictly-causal, computed transposed,
    staged bf16 in SBUF), y_std = scoresT^T @ V accumulated in PSUM
  - y_mem = Qm @ M0 with per-row LayerNorm fused via ACT scale/bias
  - delta-rule scan in 16 chunks of 128: R = K M; U = A (V - R);
    M += K^T U, with M resident in PSUM fp32 across the scan.
Host prep (cheap, O(T*N) / O(T*D)): RoPE tables + rotation of Q, the two
l2-normalized projections, beta/sigmoid, per-chunk triangular solve operators
A = (I + diag(b) tril(K K^T,-1))^{-1} diag(b), layout transposes, bf16 casts.
"""
import math
import os
import sys

import numpy as np
from ml_dtypes import bfloat16

if "/opt/trn_rl_repo" not in sys.path:
    sys.path.insert(0, "/opt/trn_rl_repo")

from contextlib import ExitStack

from concourse import bacc, mybir, tile  # noqa: E402
from concourse.bass_utils import run_bass_kernel_spmd  # noqa: E402

dt = mybir.dt
AF = mybir.ActivationFunctionType
ALU = mybir.AluOpType
AXL = mybir.AxisListType

B, NH, T, N, D = 4, 8, 2048, 256, 512
THETA = 2 ** 16
TWO_PI = 2.0 * math.pi
LN_EPS = 1e-5
C = 128              # delta chunk
NCHUNK = T // C      # 16
NT = T // 128        # 16 t-tiles
HPC = 4              # heads per core
NCORES = 8

_PROGRAM = None      # (nc) cache — compile once per process


# --------------------------------------------------------------------------
# host prep
# --------------------------------------------------------------------------

def host_prep(inputs):
    Q = np.asarray(inputs["Q"], np.float32)
    V = np.asarray(inputs["V"], np.float32)
    x_raw = np.asarray(inputs["x_raw"], np.float32)
    x_next = np.asarray(inputs["x_next"], np.float32)
    Wq = np.asarray(inputs["theta_Q_w"], np.float32)
    Wk = np.asarray(inputs["theta_K_w"], np.float32)
    bw = np.asarray(inputs["beta_w"], np.float32)
    mg = np.asarray(inputs["memory_gate"], np.float32)
    M0 = np.asarray(inputs["M0"], np.float32)

    # rope -> QRT bf16 [B,NH,N,T]
    i = np.arange(N, dtype=np.float32)
    q = np.floor(i / 2.0) * 2.0
    freqs = (1.0 / (THETA ** (q / N)) / TWO_PI)
    ph = np.mod(np.arange(T, dtype=np.float32)[:, None] * freqs[None, :], 1.0) * TWO_PI
    pc, ps = np.cos(ph).astype(np.float32), np.sin(ph).astype(np.float32)
    Qe, Qo = Q[..., ::2], Q[..., 1::2]
    Qrot = np.empty_like(Q)
    Qrot[..., ::2] = -Qo
    Qrot[..., 1::2] = Qe
    QR = Q * pc + Qrot * ps
    QRT = np.ascontiguousarray(np.swapaxes(QR, -1, -2)).astype(bfloat16)

    Qm = x_raw @ Wq.T
    Qm /= np.maximum(np.linalg.norm(Qm, axis=-1, keepdims=True), 1e-12)
    Km = x_raw @ Wk.T
    Km /= np.maximum(np.linalg.norm(Km, axis=-1, keepdims=True), 1e-12)
    QmT = np.ascontiguousarray(np.swapaxes(Qm, -1, -2)).astype(bfloat16)  # [B,N,T]
    KmT = np.ascontiguousarray(np.swapaxes(Km, -1, -2)).astype(bfloat16)
    Kmn = Km.astype(bfloat16)                                            # [B,T,N]

    beta = 1.0 / (1.0 + np.exp(-(x_raw @ bw.T)))                         # [B,T,NH]
    KmC = Km.reshape(B, NCHUNK, C, N)
    S = np.einsum("bcik,bcjk->bcij", KmC, KmC)
    S_L = np.tril(S, -1)
    Ieye = np.eye(C, dtype=np.float32)
    bC = beta.reshape(B, NCHUNK, C, NH).transpose(0, 3, 1, 2)            # [B,NH,NCHUNK,C]
    Mats = Ieye[None, None, None] + bC[..., None] * S_L[:, None]
    A = np.linalg.inv(Mats) * bC[:, :, :, None, :]                       # [B,NH,NCHUNK,C,C]
    AT = np.ascontiguousarray(np.swapaxes(A, -1, -2)).astype(np.float32)

    g = (1.0 / (1.0 + np.exp(-mg.reshape(NH)))).astype(np.float32)

    masku = np.triu(np.ones((128, 128), np.float32), 1)
    ident = np.eye(128, dtype=np.float32)

    in_maps = []
    for c in range(NCORES):
        b = c // 2
        h0 = (c % 2) * HPC
        gloc = g[h0:h0 + HPC]
        in_maps.append(dict(
            qrt=np.ascontiguousarray(QRT[b, h0:h0 + HPC]).reshape(HPC, 2, 128, T),
            qmt=np.ascontiguousarray(QmT[b]).reshape(2, 128, T),
            kmt=np.ascontiguousarray(KmT[b]).reshape(2, 128, T),
            kmn=np.ascontiguousarray(Kmn[b]).reshape(NT, 128, N),
            at=np.ascontiguousarray(AT[b, h0:h0 + HPC]),                  # [4,16,128,128]
            v=np.ascontiguousarray(V[b, h0:h0 + HPC].astype(bfloat16)).reshape(HPC, NT, 128, D),
            xn=np.ascontiguousarray(x_next[b]).reshape(NT, 128, D),
            m0=np.ascontiguousarray(M0[b, h0:h0 + HPC]).reshape(HPC, 2, 128, D),
            masku=masku,
            ident=ident,
            gcol=np.broadcast_to(gloc, (128, HPC)).copy(),
            g1col=np.broadcast_to(1.0 - gloc, (128, HPC)).copy(),
        ))
    return in_maps


# --------------------------------------------------------------------------
# device program (identical on all cores)
# --------------------------------------------------------------------------

def build_program():
    nc = bacc.Bacc("TRN2", target_bir_lowering=False, debug=False,
                   num_devices=NCORES)
    bf = dt.bfloat16
    f32 = dt.float32

    qrt_d = nc.dram_tensor("qrt", [HPC, 2, 128, T], bf, kind="ExternalInput").ap()
    qmt_d = nc.dram_tensor("qmt", [2, 128, T], bf, kind="ExternalInput").ap()
    kmt_d = nc.dram_tensor("kmt", [2, 128, T], bf, kind="ExternalInput").ap()
    kmn_d = nc.dram_tensor("kmn", [NT, 128, N], bf, kind="ExternalInput").ap()
    at_d = nc.dram_tensor("at", [HPC, NCHUNK, 128, 128], f32, kind="ExternalInput").ap()
    v_d = nc.dram_tensor("v", [HPC, NT, 128, D], bf, kind="ExternalInput").ap()
    xn_d = nc.dram_tensor("xn", [NT, 128, D], f32, kind="ExternalInput").ap()
    m0_d = nc.dram_tensor("m0", [HPC, 2, 128, D], f32, kind="ExternalInput").ap()
    masku_d = nc.dram_tensor("masku", [128, 128], f32, kind="ExternalInput").ap()
    ident_d = nc.dram_tensor("ident", [128, 128], f32, kind="ExternalInput").ap()
    gcol_d = nc.dram_tensor("gcol", [128, HPC], f32, kind="ExternalInput").ap()
    g1col_d = nc.dram_tensor("g1col", [128, HPC], f32, kind="ExternalInput").ap()
    y_d = nc.dram_tensor("y", [HPC, NT, 128, D], f32, kind="ExternalOutput").ap()
    mnew_d = nc.dram_tensor("mnew", [HPC, 2, 128, D], f32, kind="ExternalOutput").ap()

    with tile.TileContext(nc) as tc, ExitStack() as ctx:
        cst = ctx.enter_context(tc.tile_pool(name="cst", bufs=1))
        qrtp = ctx.enter_context(tc.tile_pool(name="qrtp", bufs=4))
        vp = ctx.enter_context(tc.tile_pool(name="vp", bufs=20))
        stp = ctx.enter_context(tc.tile_pool(name="stp", bufs=18))
        ymgp = ctx.enter_context(tc.tile_pool(name="ymgp", bufs=18))
        wk = ctx.enter_context(tc.tile_pool(name="wk", bufs=2))      # misc working tiles
        m0p = ctx.enter_context(tc.tile_pool(name="m0p", bufs=4))
        atp = ctx.enter_context(tc.tile_pool(name="atp", bufs=3))
        outp = ctx.enter_context(tc.tile_pool(name="outp", bufs=3))
        mmps = ctx.enter_context(tc.tile_pool(name="mmps", bufs=4, space="PSUM"))
        yps = ctx.enter_context(tc.tile_pool(name="yps", bufs=2, space="PSUM"))
        mps = ctx.enter_context(tc.tile_pool(name="mps", bufs=2, space="PSUM"))

        # ---- persistent constants / per-core tensors
        masku_sb = cst.tile([128, 128], f32, tag="masku")
        ident_sb = cst.tile([128, 128], f32, tag="ident")
        gcol_sb = cst.tile([128, HPC], f32, tag="gcol")
        g1col_sb = cst.tile([128, HPC], f32, tag="g1col")
        zero_sb = cst.tile([128, 1], f32, tag="zero")
        nc.sync.dma_start(masku_sb[:], masku_d[:])
        nc.sync.dma_start(ident_sb[:], ident_d[:])
        nc.sync.dma_start(gcol_sb[:], gcol_d[:])
        nc.sync.dma_start(g1col_sb[:], g1col_d[:])
        nc.gpsimd.memset(zero_sb[:], 0.0)

        qmt_sb = [cst.tile([128, T], bf, tag=f"qmt{i}", name=f"qmt_sb{i}") for i in range(2)]
        kmt_sb = [cst.tile([128, T], bf, tag=f"kmt{i}", name=f"kmt_sb{i}") for i in range(2)]
        for i in range(2):
            nc.sync.dma_start(qmt_sb[i][:], qmt_d[i])
            nc.sync.dma_start(kmt_sb[i][:], kmt_d[i])
        kmn_sb = [cst.tile([128, N], bf, tag=f"kmn{i}", name=f"kmn_sb{i}") for i in range(NT)]
        xn_sb = [cst.tile([128, D], f32, tag=f"xn{i}", name=f"xn_sb{i}") for i in range(NT)]
        for i in range(NT):
            nc.sync.dma_start(kmn_sb[i][:], kmn_d[i])
            nc.sync.dma_start(xn_sb[i][:], xn_d[i])

        for h in range(HPC):
            gh = gcol_sb[:, h:h + 1]
            g1h = g1col_sb[:, h:h + 1]

            # ---- loads for this head
            qrt_sb = [qrtp.tile([128, T], bf, tag="qrt", name="qrt_sb") for _ in range(2)]
            for i in range(2):
                nc.sync.dma_start(qrt_sb[i][:], qrt_d[h, i])
            v_sb = [vp.tile([128, D], bf, tag="v", name="v_sb") for _ in range(NT)]
            for i in range(NT):
                nc.sync.dma_start(v_sb[i][:], v_d[h, i])
            m0_sb = [m0p.tile([128, D], f32, tag="m0", name="m0_sb") for _ in range(2)]
            m0b_sb = [m0p.tile([128, D], bf, tag="m0b", name="m0b_sb") for _ in range(2)]
            for i in range(2):
                nc.sync.dma_start(m0_sb[i][:], m0_d[h, i])
                nc.vector.tensor_copy(m0b_sb[i][:], m0_sb[i][:])

            # ---- y_memory: ym = QmT^T @ M0, stats into strips, LN+gate via ACT
            sum_st = wk.tile([128, NT], f32, tag="sum_st")
            sq_st = wk.tile([128, NT], f32, tag="sq_st")
            ym32 = [ymgp.tile([128, D], bf, tag="ym32", name="ym32_sb") for _ in range(NT)]
            for tt in range(NT):
                ym_ps = mmps.tile([128, D], f32, tag="mm")
                nc.tensor.matmul(ym_ps[:], qmt_sb[0][:, tt * 128:(tt + 1) * 128],
                                 m0b_sb[0][:], start=True, stop=False)
                nc.tensor.matmul(ym_ps[:], qmt_sb[1][:, tt * 128:(tt + 1) * 128],
                                 m0b_sb[1][:], start=False, stop=True)
                scr = wk.tile([128, D], bf, tag="scr")
                nc.scalar.activation(scr[:], ym_ps[:], AF.Square,
                                     scale=float(1.0 / math.sqrt(D)),
                                     accum_out=sq_st[:, tt:tt + 1])
                nc.vector.tensor_reduce(out=sum_st[:, tt:tt + 1], in_=ym_ps[:],
                                        axis=AXL.X, op=ALU.add)
                nc.vector.tensor_copy(ym32[tt][:], ym_ps[:])
            # strip minis: mu, var, alpha = rsqrt(var+eps), galpha, -mu*galpha
            mu_st = wk.tile([128, NT], f32, tag="mu_st")
            nc.vector.tensor_scalar_mul(mu_st[:], sum_st[:], float(1.0 / D))
            mu2_st = wk.tile([128, NT], f32, tag="mu2_st")
            nc.vector.tensor_tensor(out=mu2_st[:], in0=mu_st[:], in1=mu_st[:], op=ALU.mult)
            var_st = wk.tile([128, NT], f32, tag="var_st")
            nc.vector.tensor_tensor(out=var_st[:], in0=sq_st[:], in1=mu2_st[:], op=ALU.subtract)
            nc.vector.tensor_scalar_add(var_st[:], var_st[:], float(LN_EPS))
            sd_st = wk.tile([128, NT], f32, tag="sd_st")
            nc.scalar.activation(sd_st[:], var_st[:], AF.Sqrt)
            al_st = wk.tile([128, NT], f32, tag="al_st")
            nc.vector.reciprocal(al_st[:], sd_st[:])
            gal_st = wk.tile([128, NT], f32, tag="gal_st")
            nc.vector.tensor_scalar_mul(gal_st[:], al_st[:], gh)
            ngm_st = wk.tile([128, NT], f32, tag="ngm_st")
            nc.vector.scalar_tensor_tensor(out=ngm_st[:], in0=mu_st[:], scalar=-1.0,
                                           in1=gal_st[:], op0=ALU.mult, op1=ALU.mult)
            ymg = [ymgp.tile([128, D], bf, tag="ymg", name="ymg_sb") for _ in range(NT)]
            for tt in range(NT):
                nc.scalar.activation(ymg[tt][:], ym32[tt][:], AF.Identity,
                                     scale=gal_st[:, tt:tt + 1],
                                     bias=ngm_st[:, tt:tt + 1])

            # ---- attention + delta, interleaved per G group
            m_ps = [mps.tile([128, D], f32, tag="mps", name="m_ps") for _ in range(2)]
            for i in range(2):
                nc.tensor.matmul(m_ps[i][:], ident_sb[:], m0_sb[i][:],
                                 start=True, stop=False)

            for G in range(4):
                t0 = G * 512
                # scoresT generation for this G window
                sT = {}
                for J in range(4 * G + 4):
                    sc_ps = mmps.tile([128, 512], f32, tag="mm")
                    nc.tensor.matmul(sc_ps[:], qrt_sb[0][:, J * 128:(J + 1) * 128],
                                     qrt_sb[0][:, t0:t0 + 512], start=True, stop=False)
                    nc.tensor.matmul(sc_ps[:], qrt_sb[1][:, J * 128:(J + 1) * 128],
                                     qrt_sb[1][:, t0:t0 + 512], start=False, stop=True)
                    st_sb = stp.tile([128, 512], bf, tag="st")
                    off = J * 128 - t0
                    if off >= 0:
                        # in-window: [0,off) zero, diag block masked, rest scaled
                        if off > 0:
                            nc.gpsimd.memset(st_sb[:, 0:off], 0.0)
                        nc.vector.scalar_tensor_tensor(
                            out=st_sb[:, off:off + 128], in0=sc_ps[:, off:off + 128],
                            scalar=g1h, in1=masku_sb[:], op0=ALU.mult, op1=ALU.mult)
                        if off + 128 < 512:
                            nc.vector.tensor_scalar_mul(
                                st_sb[:, off + 128:512], sc_ps[:, off + 128:512], g1h)
                    else:
                        if J % 2 == 0:
                            nc.vector.tensor_scalar_mul(st_sb[:], sc_ps[:], g1h)
                        else:
                            nc.scalar.activation(st_sb[:], sc_ps[:], AF.Identity,
                                                 scale=g1h, bias=zero_sb[:])
                    sT[J] = st_sb
                # y accumulation for the 4 t-tiles in this G
                for I in range(4 * G, 4 * G + 4):
                    y_ps = yps.tile([128, D], f32, tag="yps")
                    off = I * 128 - t0
                    for J in range(I + 1):
                        nc.tensor.matmul(y_ps[:], sT[J][:, off:off + 128], v_sb[J][:],
                                         start=(J == 0), stop=(J == I))
                    y_out = outp.tile([128, D], f32, tag="y_out")
                    nc.vector.tensor_tensor(out=y_out[:], in0=y_ps[:],
                                            in1=ymg[I][:], op=ALU.add)
                    nc.sync.dma_start(y_d[h, I], y_out[:])

                # 4 delta chunks interleaved after each G
                for cc in range(4 * G, 4 * G + 4):
                    at_sb = atp.tile([128, 128], f32, tag="at")
                    nc.sync.dma_start(at_sb[:], at_d[h, cc])
                    msb = [wk.tile([128, D], bf, tag=f"msb{i}", name=f"msb{i}") for i in range(2)]
                    for i in range(2):
                        nc.scalar.activation(msb[i][:], m_ps[i][:], AF.Copy)
                    r_ps = mmps.tile([128, D], f32, tag="mm")
                    nc.tensor.matmul(r_ps[:], kmt_sb[0][:, cc * 128:(cc + 1) * 128],
                                     msb[0][:], start=True, stop=False)
                    nc.tensor.matmul(r_ps[:], kmt_sb[1][:, cc * 128:(cc + 1) * 128],
                                     msb[1][:], start=False, stop=True)
                    u_in = wk.tile([128, D], f32, tag="u_in")
                    nc.vector.tensor_tensor(out=u_in[:], in0=xn_sb[cc][:],
                                            in1=r_ps[:], op=ALU.subtract)
                    u_ps = mmps.tile([128, D], f32, tag="mm")
                    nc.tensor.matmul(u_ps[:], at_sb[:], u_in[:], start=True, stop=True)
                    u_sb = wk.tile([128, D], bf, tag="u_sb")
                    nc.vector.tensor_copy(u_sb[:], u_ps[:])
                    last = (cc == NCHUNK - 1)
                    for i in range(2):
                        nc.tensor.matmul(m_ps[i][:],
                                         kmn_sb[cc][:, i * 128:(i + 1) * 128],
                                         u_sb[:], start=False, stop=last)

            # ---- M_new evacuation
            for i in range(2):
                mn_sb = outp.tile([128, D], f32, tag="mn")
                nc.vector.tensor_copy(mn_sb[:], m_ps[i][:])
                nc.sync.dma_start(mnew_d[h, i], mn_sb[:])

    nc.compile()
    return nc


def _get_program():
    global _PROGRAM
    if _PROGRAM is None:
        _PROGRAM = build_program()
    return _PROGRAM


# --------------------------------------------------------------------------
# public entry
# --------------------------------------------------------------------------

def kernel(**inputs):
    nc = _get_program()
    in_maps = host_prep(inputs)
    res = run_bass_kernel_spmd(nc, in_maps, list(range(NCORES)))
    y = np.zeros((B, NH, T, D), np.float32)
    M_new = np.zeros((B, NH, N, D), np.float32)
    for c in range(NCORES):
        b = c // 2
        h0 = (c % 2) * HPC
        yc = res.results[c]["y"].reshape(HPC, T, D)
        mc = res.results[c]["mnew"].reshape(HPC, N, D)
        y[b, h0:h0 + HPC] = yc
        M_new[b, h0:h0 + HPC] = mc
    return y, M_new


def run_profiled(inputs):
    """Like kernel() but with NTFF tracing; returns (y, M_new, exec_time_ns)."""
    nc = _get_program()
    in_maps = host_prep(inputs)
    res = run_bass_kernel_spmd(nc, in_maps, list(range(NCORES)),
                               trace=True, trace_cores=[0])
    y = np.zeros((B, NH, T, D), np.float32)
    M_new = np.zeros((B, NH, N, D), np.float32)
    for c in range(NCORES):
        b = c // 2
        h0 = (c % 2) * HPC
        y[b, h0:h0 + HPC] = res.results[c]["y"].reshape(HPC, T, D)
        M_new[b, h0:h0 + HPC] = res.results[c]["mnew"].reshape(HPC, N, D)
    return y, M_new, res.exec_time_ns


# revision 3
# speedup vs baseline: 1.2568x; 1.2568x over previous
"""Trainium2 Bass kernel for nn_Attention_4612794875918.

Full inputs in, full outputs out. Internally shards across 8 NeuronCores:
core c handles batch b = c//2, head group hg = c%2 (4 heads each) — scores,
y_standard, memory matrix M and the delta scan are independent per (B, nh).

Device does all O(T^2) / O(T*N*D) matmul work per (b,h):
  - scoresT = rope(Q) @ rope(Q)^T (strictly-causal, computed transposed,
    staged bf16 in SBUF), y_std = scoresT^T @ V accumulated in PSUM
  - y_mem = Qm @ M0 with per-row LayerNorm fused via ACT scale/bias
  - delta-rule scan in 16 chunks of 128: R = K M; U = A (V - R);
    M += K^T U, with M resident in PSUM fp32 across the scan.
Host prep (cheap, O(T*N) / O(T*D)): RoPE tables + rotation of Q, the two
l2-normalized projections, beta/sigmoid, per-chunk triangular solve operators
A = (I + diag(b) tril(K K^T,-1))^{-1} diag(b), layout transposes, bf16 casts.
"""
import math
import os
import sys

import numpy as np
from ml_dtypes import bfloat16

if "/opt/trn_rl_repo" not in sys.path:
    sys.path.insert(0, "/opt/trn_rl_repo")

from contextlib import ExitStack

from concourse import bacc, mybir, tile  # noqa: E402
from concourse.bass_utils import run_bass_kernel_spmd  # noqa: E402

dt = mybir.dt
AF = mybir.ActivationFunctionType
ALU = mybir.AluOpType
AXL = mybir.AxisListType

B, NH, T, N, D = 4, 8, 2048, 256, 512
THETA = 2 ** 16
TWO_PI = 2.0 * math.pi
LN_EPS = 1e-5
C = 128              # delta chunk
NCHUNK = T // C      # 16
NT = T // 128        # 16 t-tiles
HPC = 4              # heads per core
NCORES = 8

_PROGRAM = None      # (nc) cache — compile once per process


# --------------------------------------------------------------------------
# host prep
# --------------------------------------------------------------------------

def host_prep(inputs):
    Q = np.asarray(inputs["Q"], np.float32)
    V = np.asarray(inputs["V"], np.float32)
    x_raw = np.asarray(inputs["x_raw"], np.float32)
    x_next = np.asarray(inputs["x_next"], np.float32)
    Wq = np.asarray(inputs["theta_Q_w"], np.float32)
    Wk = np.asarray(inputs["theta_K_w"], np.float32)
    bw = np.asarray(inputs["beta_w"], np.float32)
    mg = np.asarray(inputs["memory_gate"], np.float32)
    M0 = np.asarray(inputs["M0"], np.float32)

    # rope -> QRT bf16 [B,NH,N,T]
    i = np.arange(N, dtype=np.float32)
    q = np.floor(i / 2.0) * 2.0
    freqs = (1.0 / (THETA ** (q / N)) / TWO_PI)
    ph = np.mod(np.arange(T, dtype=np.float32)[:, None] * freqs[None, :], 1.0) * TWO_PI
    pc, ps = np.cos(ph).astype(np.float32), np.sin(ph).astype(np.float32)
    Qe, Qo = Q[..., ::2], Q[..., 1::2]
    Qrot = np.empty_like(Q)
    Qrot[..., ::2] = -Qo
    Qrot[..., 1::2] = Qe
    QR = Q * pc + Qrot * ps
    QRT = np.ascontiguousarray(np.swapaxes(QR, -1, -2)).astype(bfloat16)

    Qm = x_raw @ Wq.T
    Qm /= np.maximum(np.linalg.norm(Qm, axis=-1, keepdims=True), 1e-12)
    Km = x_raw @ Wk.T
    Km /= np.maximum(np.linalg.norm(Km, axis=-1, keepdims=True), 1e-12)
    QmT = np.ascontiguousarray(np.swapaxes(Qm, -1, -2)).astype(bfloat16)  # [B,N,T]
    KmT = np.ascontiguousarray(np.swapaxes(Km, -1, -2)).astype(bfloat16)
    Kmn = Km.astype(bfloat16)                                            # [B,T,N]

    beta = 1.0 / (1.0 + np.exp(-(x_raw @ bw.T)))                         # [B,T,NH]
    KmC = Km.reshape(B, NCHUNK, C, N)
    S = np.einsum("bcik,bcjk->bcij", KmC, KmC)
    S_L = np.tril(S, -1)
    Ieye = np.eye(C, dtype=np.float32)
    bC = beta.reshape(B, NCHUNK, C, NH).transpose(0, 3, 1, 2)            # [B,NH,NCHUNK,C]
    Mats = Ieye[None, None, None] + bC[..., None] * S_L[:, None]
    A = np.linalg.inv(Mats) * bC[:, :, :, None, :]                       # [B,NH,NCHUNK,C,C]
    AT = np.ascontiguousarray(np.swapaxes(A, -1, -2)).astype(np.float32)

    g = (1.0 / (1.0 + np.exp(-mg.reshape(NH)))).astype(np.float32)

    masku = np.triu(np.ones((128, 128), np.float32), 1)
    ident = np.eye(128, dtype=np.float32).astype(bfloat16)

    in_maps = []
    for c in range(NCORES):
        b = c // 2
        h0 = (c % 2) * HPC
        gloc = g[h0:h0 + HPC]
        in_maps.append(dict(
            qrt=np.ascontiguousarray(QRT[b, h0:h0 + HPC]).reshape(HPC, 2, 128, T),
            qmt=np.ascontiguousarray(QmT[b]).reshape(2, 128, T),
            kmt=np.ascontiguousarray(KmT[b]).reshape(2, 128, T),
            kmn=np.ascontiguousarray(Kmn[b]).reshape(NT, 128, N),
            at=np.ascontiguousarray(AT[b, h0:h0 + HPC]).astype(bfloat16),  # [4,16,128,128]
            v=np.ascontiguousarray(V[b, h0:h0 + HPC].astype(bfloat16)).reshape(HPC, NT, 128, D),
            xn=np.ascontiguousarray(x_next[b]).reshape(NT, 128, D),
            m0=np.ascontiguousarray(M0[b, h0:h0 + HPC].astype(bfloat16)).reshape(HPC, 2, 128, D),
            masku=masku,
            ident=ident,
            gcol=np.broadcast_to(gloc, (128, HPC)).copy(),
            g1col=np.broadcast_to(1.0 - gloc, (128, HPC)).copy(),
        ))
    return in_maps


# --------------------------------------------------------------------------
# device program (identical on all cores)
# --------------------------------------------------------------------------

def build_program():
    nc = bacc.Bacc("TRN2", target_bir_lowering=False, debug=False,
                   num_devices=NCORES)
    bf = dt.bfloat16
    f32 = dt.float32

    qrt_d = nc.dram_tensor("qrt", [HPC, 2, 128, T], bf, kind="ExternalInput").ap()
    qmt_d = nc.dram_tensor("qmt", [2, 128, T], bf, kind="ExternalInput").ap()
    kmt_d = nc.dram_tensor("kmt", [2, 128, T], bf, kind="ExternalInput").ap()
    kmn_d = nc.dram_tensor("kmn", [NT, 128, N], bf, kind="ExternalInput").ap()
    at_d = nc.dram_tensor("at", [HPC, NCHUNK, 128, 128], bf, kind="ExternalInput").ap()
    v_d = nc.dram_tensor("v", [HPC, NT, 128, D], bf, kind="ExternalInput").ap()
    xn_d = nc.dram_tensor("xn", [NT, 128, D], f32, kind="ExternalInput").ap()
    m0_d = nc.dram_tensor("m0", [HPC, 2, 128, D], bf, kind="ExternalInput").ap()
    masku_d = nc.dram_tensor("masku", [128, 128], f32, kind="ExternalInput").ap()
    ident_d = nc.dram_tensor("ident", [128, 128], bf, kind="ExternalInput").ap()
    gcol_d = nc.dram_tensor("gcol", [128, HPC], f32, kind="ExternalInput").ap()
    g1col_d = nc.dram_tensor("g1col", [128, HPC], f32, kind="ExternalInput").ap()
    y_d = nc.dram_tensor("y", [HPC, NT, 128, D], f32, kind="ExternalOutput").ap()
    mnew_d = nc.dram_tensor("mnew", [HPC, 2, 128, D], f32, kind="ExternalOutput").ap()

    with tile.TileContext(nc) as tc, ExitStack() as ctx:
        cst = ctx.enter_context(tc.tile_pool(name="cst", bufs=1))
        qrtp = ctx.enter_context(tc.tile_pool(name="qrtp", bufs=4))
        vp = ctx.enter_context(tc.tile_pool(name="vp", bufs=20))
        stp = ctx.enter_context(tc.tile_pool(name="stp", bufs=18))
        ymgp = ctx.enter_context(tc.tile_pool(name="ymgp", bufs=18))
        wk = ctx.enter_context(tc.tile_pool(name="wk", bufs=2))      # misc working tiles
        m0p = ctx.enter_context(tc.tile_pool(name="m0p", bufs=4))
        atp = ctx.enter_context(tc.tile_pool(name="atp", bufs=3))
        outp = ctx.enter_context(tc.tile_pool(name="outp", bufs=3))
        mmps = ctx.enter_context(tc.tile_pool(name="mmps", bufs=4, space="PSUM"))
        yps = ctx.enter_context(tc.tile_pool(name="yps", bufs=2, space="PSUM"))
        mps = ctx.enter_context(tc.tile_pool(name="mps", bufs=2, space="PSUM"))

        # ---- persistent constants / per-core tensors
        masku_sb = cst.tile([128, 128], f32, tag="masku")
        ident_sb = cst.tile([128, 128], bf, tag="ident")
        gcol_sb = cst.tile([128, HPC], f32, tag="gcol")
        g1col_sb = cst.tile([128, HPC], f32, tag="g1col")
        zero_sb = cst.tile([128, 1], f32, tag="zero")
        nc.sync.dma_start(masku_sb[:], masku_d[:])
        nc.sync.dma_start(ident_sb[:], ident_d[:])
        nc.sync.dma_start(gcol_sb[:], gcol_d[:])
        nc.sync.dma_start(g1col_sb[:], g1col_d[:])
        nc.gpsimd.memset(zero_sb[:], 0.0)

        qmt_sb = [cst.tile([128, T], bf, tag=f"qmt{i}", name=f"qmt_sb{i}") for i in range(2)]
        kmt_sb = [cst.tile([128, T], bf, tag=f"kmt{i}", name=f"kmt_sb{i}") for i in range(2)]
        for i in range(2):
            nc.sync.dma_start(qmt_sb[i][:], qmt_d[i])
            nc.sync.dma_start(kmt_sb[i][:], kmt_d[i])
        kmn_sb = [cst.tile([128, N], bf, tag=f"kmn{i}", name=f"kmn_sb{i}") for i in range(NT)]
        xn_sb = [cst.tile([128, D], f32, tag=f"xn{i}", name=f"xn_sb{i}") for i in range(NT)]
        for i in range(NT):
            nc.sync.dma_start(kmn_sb[i][:], kmn_d[i])
            nc.sync.dma_start(xn_sb[i][:], xn_d[i])

        for h in range(HPC):
            gh = gcol_sb[:, h:h + 1]
            g1h = g1col_sb[:, h:h + 1]

            # ---- loads for this head
            qrt_sb = [qrtp.tile([128, T], bf, tag="qrt", name="qrt_sb") for _ in range(2)]
            for i in range(2):
                nc.sync.dma_start(qrt_sb[i][:], qrt_d[h, i])
            v_sb = [vp.tile([128, D], bf, tag="v", name="v_sb") for _ in range(NT)]
            for i in range(NT):
                nc.sync.dma_start(v_sb[i][:], v_d[h, i])
            m0b_sb = [m0p.tile([128, D], bf, tag="m0b", name="m0b_sb") for _ in range(2)]
            for i in range(2):
                nc.sync.dma_start(m0b_sb[i][:], m0_d[h, i])

            # ---- y_memory: ym = QmT^T @ M0, stats into strips, LN+gate via ACT
            sum_st = wk.tile([128, NT], f32, tag="sum_st")
            sq_st = wk.tile([128, NT], f32, tag="sq_st")
            ym32 = [ymgp.tile([128, D], bf, tag="ym32", name="ym32_sb") for _ in range(NT)]
            for tt in range(NT):
                ym_ps = mmps.tile([128, D], f32, tag="mm")
                nc.tensor.matmul(ym_ps[:], qmt_sb[0][:, tt * 128:(tt + 1) * 128],
                                 m0b_sb[0][:], start=True, stop=False)
                nc.tensor.matmul(ym_ps[:], qmt_sb[1][:, tt * 128:(tt + 1) * 128],
                                 m0b_sb[1][:], start=False, stop=True)
                scr = wk.tile([128, D], bf, tag="scr")
                nc.scalar.activation(scr[:], ym_ps[:], AF.Square,
                                     scale=float(1.0 / math.sqrt(D)),
                                     accum_out=sq_st[:, tt:tt + 1])
                nc.scalar.activation(ym32[tt][:], ym_ps[:], AF.Copy,
                                     accum_out=sum_st[:, tt:tt + 1])
            # strip minis: mu, var, alpha = rsqrt(var+eps), galpha, -mu*galpha
            mu_st = wk.tile([128, NT], f32, tag="mu_st")
            nc.vector.tensor_scalar_mul(mu_st[:], sum_st[:], float(1.0 / D))
            mu2_st = wk.tile([128, NT], f32, tag="mu2_st")
            nc.vector.tensor_tensor(out=mu2_st[:], in0=mu_st[:], in1=mu_st[:], op=ALU.mult)
            var_st = wk.tile([128, NT], f32, tag="var_st")
            nc.vector.tensor_tensor(out=var_st[:], in0=sq_st[:], in1=mu2_st[:], op=ALU.subtract)
            nc.vector.tensor_scalar_add(var_st[:], var_st[:], float(LN_EPS))
            sd_st = wk.tile([128, NT], f32, tag="sd_st")
            nc.scalar.activation(sd_st[:], var_st[:], AF.Sqrt)
            al_st = wk.tile([128, NT], f32, tag="al_st")
            nc.vector.reciprocal(al_st[:], sd_st[:])
            gal_st = wk.tile([128, NT], f32, tag="gal_st")
            nc.vector.tensor_scalar_mul(gal_st[:], al_st[:], gh)
            ngm_st = wk.tile([128, NT], f32, tag="ngm_st")
            nc.vector.scalar_tensor_tensor(out=ngm_st[:], in0=mu_st[:], scalar=-1.0,
                                           in1=gal_st[:], op0=ALU.mult, op1=ALU.mult)
            ymg = [ymgp.tile([128, D], bf, tag="ymg", name="ymg_sb") for _ in range(NT)]
            for tt in range(NT):
                nc.scalar.activation(ymg[tt][:], ym32[tt][:], AF.Identity,
                                     scale=gal_st[:, tt:tt + 1],
                                     bias=ngm_st[:, tt:tt + 1])

            # ---- attention + delta, interleaved per G group
            m_ps = [mps.tile([128, D], f32, tag="mps", name="m_ps") for _ in range(2)]
            for i in range(2):
                nc.tensor.matmul(m_ps[i][:], ident_sb[:], m0b_sb[i][:],
                                 start=True, stop=False)

            for G in range(4):
                t0 = G * 512
                # scoresT generation for this G window
                sT = {}
                for J in range(4 * G + 4):
                    sc_ps = mmps.tile([128, 512], f32, tag="mm")
                    nc.tensor.matmul(sc_ps[:], qrt_sb[0][:, J * 128:(J + 1) * 128],
                                     qrt_sb[0][:, t0:t0 + 512], start=True, stop=False)
                    nc.tensor.matmul(sc_ps[:], qrt_sb[1][:, J * 128:(J + 1) * 128],
                                     qrt_sb[1][:, t0:t0 + 512], start=False, stop=True)
                    st_sb = stp.tile([128, 512], bf, tag="st")
                    off = J * 128 - t0
                    if off >= 0:
                        # in-window: [0,off) zero, diag block masked, rest scaled
                        if off > 0:
                            nc.gpsimd.memset(st_sb[:, 0:off], 0.0)
                        nc.vector.scalar_tensor_tensor(
                            out=st_sb[:, off:off + 128], in0=sc_ps[:, off:off + 128],
                            scalar=g1h, in1=masku_sb[:], op0=ALU.mult, op1=ALU.mult)
                        if off + 128 < 512:
                            nc.vector.tensor_scalar_mul(
                                st_sb[:, off + 128:512], sc_ps[:, off + 128:512], g1h)
                    else:
                        if J % 2 == 0:
                            nc.vector.tensor_scalar_mul(st_sb[:], sc_ps[:], g1h)
                        else:
                            nc.scalar.activation(st_sb[:], sc_ps[:], AF.Identity,
                                                 scale=g1h, bias=zero_sb[:])
                    sT[J] = st_sb
                # y accumulation + one delta chunk per t-tile in this G
                for I in range(4 * G, 4 * G + 4):
                    y_ps = yps.tile([128, D], f32, tag="yps")
                    off = I * 128 - t0
                    for J in range(I + 1):
                        nc.tensor.matmul(y_ps[:], sT[J][:, off:off + 128], v_sb[J][:],
                                         start=(J == 0), stop=(J == I))
                    y_out = outp.tile([128, D], f32, tag="y_out")
                    nc.vector.tensor_tensor(out=y_out[:], in0=y_ps[:],
                                            in1=ymg[I][:], op=ALU.add)
                    nc.sync.dma_start(y_d[h, I], y_out[:])

                    cc = I
                    at_sb = atp.tile([128, 128], bf, tag="at")
                    nc.sync.dma_start(at_sb[:], at_d[h, cc])
                    msb = [wk.tile([128, D], bf, tag=f"msb{i}", name=f"msb{i}") for i in range(2)]
                    nc.scalar.activation(msb[0][:], m_ps[0][:], AF.Copy)
                    nc.vector.tensor_copy(msb[1][:], m_ps[1][:])
                    r_ps = mmps.tile([128, D], f32, tag="mm")
                    nc.tensor.matmul(r_ps[:], kmt_sb[0][:, cc * 128:(cc + 1) * 128],
                                     msb[0][:], start=True, stop=False)
                    nc.tensor.matmul(r_ps[:], kmt_sb[1][:, cc * 128:(cc + 1) * 128],
                                     msb[1][:], start=False, stop=True)
                    u_in = wk.tile([128, D], bf, tag="u_in")
                    nc.vector.tensor_tensor(out=u_in[:], in0=xn_sb[cc][:],
                                            in1=r_ps[:], op=ALU.subtract)
                    u_ps = mmps.tile([128, D], f32, tag="mm")
                    nc.tensor.matmul(u_ps[:], at_sb[:], u_in[:], start=True, stop=True)
                    u_sb = wk.tile([128, D], bf, tag="u_sb")
                    nc.vector.tensor_copy(u_sb[:], u_ps[:])
                    last = (cc == NCHUNK - 1)
                    for i in range(2):
                        nc.tensor.matmul(m_ps[i][:],
                                         kmn_sb[cc][:, i * 128:(i + 1) * 128],
                                         u_sb[:], start=False, stop=last)

            # ---- M_new evacuation
            for i in range(2):
                mn_sb = outp.tile([128, D], f32, tag="mn")
                nc.vector.tensor_copy(mn_sb[:], m_ps[i][:])
                nc.sync.dma_start(mnew_d[h, i], mn_sb[:])

    nc.compile()
    return nc


def _get_program():
    global _PROGRAM
    if _PROGRAM is None:
        _PROGRAM = build_program()
    return _PROGRAM


# --------------------------------------------------------------------------
# public entry
# --------------------------------------------------------------------------

def kernel(**inputs):
    nc = _get_program()
    in_maps = host_prep(inputs)
    res = run_bass_kernel_spmd(nc, in_maps, list(range(NCORES)))
    y = np.zeros((B, NH, T, D), np.float32)
    M_new = np.zeros((B, NH, N, D), np.float32)
    for c in range(NCORES):
        b = c // 2
        h0 = (c % 2) * HPC
        yc = res.results[c]["y"].reshape(HPC, T, D)
        mc = res.results[c]["mnew"].reshape(HPC, N, D)
        y[b, h0:h0 + HPC] = yc
        M_new[b, h0:h0 + HPC] = mc
    return y, M_new


def run_profiled(inputs):
    """Like kernel() but with NTFF tracing; returns (y, M_new, exec_time_ns)."""
    nc = _get_program()
    in_maps = host_prep(inputs)
    res = run_bass_kernel_spmd(nc, in_maps, list(range(NCORES)),
                               trace=True, trace_cores=[0])
    y = np.zeros((B, NH, T, D), np.float32)
    M_new = np.zeros((B, NH, N, D), np.float32)
    for c in range(NCORES):
        b = c // 2
        h0 = (c % 2) * HPC
        y[b, h0:h0 + HPC] = res.results[c]["y"].reshape(HPC, T, D)
        M_new[b, h0:h0 + HPC] = res.results[c]["mnew"].reshape(HPC, N, D)
    return y, M_new, res.exec_time_ns


# revision 5
# speedup vs baseline: 1.3458x; 1.0708x over previous
"""Trainium2 Bass kernel for nn_Attention_4612794875918.

Full inputs in, full outputs out. Internally shards across 8 NeuronCores:
core c handles batch b = c//2, head group hg = c%2 (4 heads each) — scores,
y_standard, memory matrix M and the delta scan are independent per (B, nh).

Device does all O(T^2) / O(T*N*D) matmul work per (b,h):
  - scoresT = rope(Q) @ rope(Q)^T (strictly-causal, computed transposed,
    staged bf16 in SBUF), y_std = scoresT^T @ V accumulated in PSUM
  - y_mem = Qm @ M0 with per-row LayerNorm fused via ACT scale/bias
  - delta-rule scan in 16 chunks of 128 with a 2-chunk-lagged state copy:
    R_c = K_c M_(c-2) + G2_c U_(c-2) + G1_c U_(c-1);  U_c = A_c (V_c - R_c);
    M += K_c^T U_c, M resident in PSUM fp32 across the scan. The lag keeps
    the PSUM->SBUF state copy off the PE critical path.
Host prep (cheap, O(T*N) / O(T*D)): RoPE tables + rotation of Q, the two
l2-normalized projections, beta/sigmoid, per-chunk triangular solve operators
A = (I + diag(b) tril(K K^T,-1))^{-1} diag(b), the chunk-coupling Gram
matrices G1/G2, layout transposes, bf16 casts.
"""
import math
import sys

import numpy as np
from ml_dtypes import bfloat16

if "/opt/trn_rl_repo" not in sys.path:
    sys.path.insert(0, "/opt/trn_rl_repo")

from contextlib import ExitStack

from concourse import bacc, mybir, tile  # noqa: E402
from concourse.bass_utils import run_bass_kernel_spmd  # noqa: E402

dt = mybir.dt
AF = mybir.ActivationFunctionType
ALU = mybir.AluOpType
AXL = mybir.AxisListType

B, NH, T, N, D = 4, 8, 2048, 256, 512
THETA = 2 ** 16
TWO_PI = 2.0 * math.pi
LN_EPS = 1e-5
C = 128              # delta chunk
NCHUNK = T // C      # 16
NT = T // 128        # 16 t-tiles
HPC = 4              # heads per core
NCORES = 8

_PROGRAM = None      # compile once per process


# --------------------------------------------------------------------------
# host prep
# --------------------------------------------------------------------------

def host_prep(inputs):
    Q = np.asarray(inputs["Q"], np.float32)
    V = np.asarray(inputs["V"], np.float32)
    x_raw = np.asarray(inputs["x_raw"], np.float32)
    x_next = np.asarray(inputs["x_next"], np.float32)
    Wq = np.asarray(inputs["theta_Q_w"], np.float32)
    Wk = np.asarray(inputs["theta_K_w"], np.float32)
    bw = np.asarray(inputs["beta_w"], np.float32)
    mg = np.asarray(inputs["memory_gate"], np.float32)
    M0 = np.asarray(inputs["M0"], np.float32)

    # rope -> QRT bf16 [B,NH,N,T]
    i = np.arange(N, dtype=np.float32)
    q = np.floor(i / 2.0) * 2.0
    freqs = (1.0 / (THETA ** (q / N)) / TWO_PI)
    ph = np.mod(np.arange(T, dtype=np.float32)[:, None] * freqs[None, :], 1.0) * TWO_PI
    pc, ps = np.cos(ph).astype(np.float32), np.sin(ph).astype(np.float32)
    Qe, Qo = Q[..., ::2], Q[..., 1::2]
    Qrot = np.empty_like(Q)
    Qrot[..., ::2] = -Qo
    Qrot[..., 1::2] = Qe
    QR = Q * pc + Qrot * ps
    QRT = np.ascontiguousarray(np.swapaxes(QR, -1, -2)).astype(bfloat16)

    Qm = x_raw @ Wq.T
    Qm /= np.maximum(np.linalg.norm(Qm, axis=-1, keepdims=True), 1e-12)
    Km = x_raw @ Wk.T
    Km /= np.maximum(np.linalg.norm(Km, axis=-1, keepdims=True), 1e-12)
    QmT = np.ascontiguousarray(np.swapaxes(Qm, -1, -2)).astype(bfloat16)  # [B,N,T]
    KmT = np.ascontiguousarray(np.swapaxes(Km, -1, -2)).astype(bfloat16)
    Kmn = Km.astype(bfloat16)                                            # [B,T,N]

    beta = 1.0 / (1.0 + np.exp(-(x_raw @ bw.T)))                         # [B,T,NH]
    KmC = Km.reshape(B, NCHUNK, C, N)
    S = np.einsum("bcik,bcjk->bcij", KmC, KmC)
    S_L = np.tril(S, -1)
    Ieye = np.eye(C, dtype=np.float32)
    bC = beta.reshape(B, NCHUNK, C, NH).transpose(0, 3, 1, 2)            # [B,NH,NCHUNK,C]
    Mats = Ieye[None, None, None] + bC[..., None] * S_L[:, None]
    A = np.linalg.inv(Mats) * bC[:, :, :, None, :]                       # [B,NH,NCHUNK,C,C]
    AT = np.ascontiguousarray(np.swapaxes(A, -1, -2)).astype(bfloat16)

    # chunk-coupling Gram matrices in lhsT form:
    # g1t[c] = Km_(c-1) Km_c^T (c>=1), g2t[c] = Km_(c-2) Km_c^T (c>=2)
    g1t = np.zeros((B, NCHUNK, C, C), np.float32)
    g2t = np.zeros((B, NCHUNK, C, C), np.float32)
    for b in range(B):
        for c in range(1, NCHUNK):
            g1t[b, c] = KmC[b, c - 1] @ KmC[b, c].T
        for c in range(2, NCHUNK):
            g2t[b, c] = KmC[b, c - 2] @ KmC[b, c].T
    g1t = g1t.astype(bfloat16)
    g2t = g2t.astype(bfloat16)

    g = (1.0 / (1.0 + np.exp(-mg.reshape(NH)))).astype(np.float32)

    masku = np.triu(np.ones((128, 128), np.float32), 1)
    ident = np.eye(128, dtype=np.float32).astype(bfloat16)

    in_maps = []
    for c in range(NCORES):
        b = c // 2
        h0 = (c % 2) * HPC
        gloc = g[h0:h0 + HPC]
        in_maps.append(dict(
            qrt=np.ascontiguousarray(QRT[b, h0:h0 + HPC]).reshape(HPC, 2, 128, T),
            qmt=np.ascontiguousarray(QmT[b]).reshape(2, 128, T),
            kmt=np.ascontiguousarray(KmT[b]).reshape(2, 128, T),
            kmn=np.ascontiguousarray(Kmn[b]).reshape(NT, 128, N),
            at=np.ascontiguousarray(AT[b, h0:h0 + HPC]),                  # [4,16,128,128] bf16
            g1t=g1t[b], g2t=g2t[b],                                       # [16,128,128] bf16
            v=np.ascontiguousarray(V[b, h0:h0 + HPC].astype(bfloat16)).reshape(HPC, NT, 128, D),
            xn=np.ascontiguousarray(x_next[b]).reshape(NT, 128, D),
            m0=np.ascontiguousarray(M0[b, h0:h0 + HPC].astype(bfloat16)).reshape(HPC, 2, 128, D),
            masku=masku,
            ident=ident,
            gcol=np.broadcast_to(gloc, (128, HPC)).copy(),
            g1col=np.broadcast_to(1.0 - gloc, (128, HPC)).copy(),
        ))
    return in_maps


# --------------------------------------------------------------------------
# device program (identical on all cores)
# --------------------------------------------------------------------------

def build_program():
    nc = bacc.Bacc("TRN2", target_bir_lowering=False, debug=False,
                   num_devices=NCORES)
    bf = dt.bfloat16
    f32 = dt.float32

    qrt_d = nc.dram_tensor("qrt", [HPC, 2, 128, T], bf, kind="ExternalInput").ap()
    qmt_d = nc.dram_tensor("qmt", [2, 128, T], bf, kind="ExternalInput").ap()
    kmt_d = nc.dram_tensor("kmt", [2, 128, T], bf, kind="ExternalInput").ap()
    kmn_d = nc.dram_tensor("kmn", [NT, 128, N], bf, kind="ExternalInput").ap()
    at_d = nc.dram_tensor("at", [HPC, NCHUNK, 128, 128], bf, kind="ExternalInput").ap()
    g1t_d = nc.dram_tensor("g1t", [NCHUNK, 128, 128], bf, kind="ExternalInput").ap()
    g2t_d = nc.dram_tensor("g2t", [NCHUNK, 128, 128], bf, kind="ExternalInput").ap()
    v_d = nc.dram_tensor("v", [HPC, NT, 128, D], bf, kind="ExternalInput").ap()
    xn_d = nc.dram_tensor("xn", [NT, 128, D], f32, kind="ExternalInput").ap()
    m0_d = nc.dram_tensor("m0", [HPC, 2, 128, D], bf, kind="ExternalInput").ap()
    masku_d = nc.dram_tensor("masku", [128, 128], f32, kind="ExternalInput").ap()
    ident_d = nc.dram_tensor("ident", [128, 128], bf, kind="ExternalInput").ap()
    gcol_d = nc.dram_tensor("gcol", [128, HPC], f32, kind="ExternalInput").ap()
    g1col_d = nc.dram_tensor("g1col", [128, HPC], f32, kind="ExternalInput").ap()
    y_d = nc.dram_tensor("y", [HPC, NT, 128, D], f32, kind="ExternalOutput").ap()
    mnew_d = nc.dram_tensor("mnew", [HPC, 2, 128, D], f32, kind="ExternalOutput").ap()

    with tile.TileContext(nc) as tc, ExitStack() as ctx:
        cst = ctx.enter_context(tc.tile_pool(name="cst", bufs=1))
        qrtp = ctx.enter_context(tc.tile_pool(name="qrtp", bufs=3))
        vp = ctx.enter_context(tc.tile_pool(name="vp", bufs=18))
        stp = ctx.enter_context(tc.tile_pool(name="stp", bufs=18))
        ymgp = ctx.enter_context(tc.tile_pool(name="ymgp", bufs=16))
        wk = ctx.enter_context(tc.tile_pool(name="wk", bufs=2))
        dl = ctx.enter_context(tc.tile_pool(name="dl", bufs=4))      # delta lagged state
        m0p = ctx.enter_context(tc.tile_pool(name="m0p", bufs=4))
        atp = ctx.enter_context(tc.tile_pool(name="atp", bufs=3))
        outp = ctx.enter_context(tc.tile_pool(name="outp", bufs=3))
        mmps = ctx.enter_context(tc.tile_pool(name="mmps", bufs=4, space="PSUM"))
        yps = ctx.enter_context(tc.tile_pool(name="yps", bufs=2, space="PSUM"))
        mps = ctx.enter_context(tc.tile_pool(name="mps", bufs=2, space="PSUM"))

        # ---- persistent constants / per-core tensors
        masku_sb = cst.tile([128, 128], f32, tag="masku")
        ident_sb = cst.tile([128, 128], bf, tag="ident")
        gcol_sb = cst.tile([128, HPC], f32, tag="gcol")
        g1col_sb = cst.tile([128, HPC], f32, tag="g1col")
        zero_sb = cst.tile([128, 1], f32, tag="zero")
        nc.sync.dma_start(masku_sb[:], masku_d[:])
        nc.sync.dma_start(ident_sb[:], ident_d[:])
        nc.sync.dma_start(gcol_sb[:], gcol_d[:])
        nc.sync.dma_start(g1col_sb[:], g1col_d[:])
        nc.gpsimd.memset(zero_sb[:], 0.0)

        qmt_sb = [cst.tile([128, T], bf, tag=f"qmt{i}", name=f"qmt_sb{i}") for i in range(2)]
        kmt_sb = [cst.tile([128, T], bf, tag=f"kmt{i}", name=f"kmt_sb{i}") for i in range(2)]
        for i in range(2):
            nc.sync.dma_start(qmt_sb[i][:], qmt_d[i])
            nc.sync.dma_start(kmt_sb[i][:], kmt_d[i])
        kmn_sb = [cst.tile([128, N], bf, tag=f"kmn{i}", name=f"kmn_sb{i}") for i in range(NT)]
        xn_sb = [cst.tile([128, D], f32, tag=f"xn{i}", name=f"xn_sb{i}") for i in range(NT)]
        g1t_sb = [cst.tile([128, 128], bf, tag=f"g1t{i}", name=f"g1t_sb{i}") for i in range(NCHUNK)]
        g2t_sb = [cst.tile([128, 128], bf, tag=f"g2t{i}", name=f"g2t_sb{i}") for i in range(NCHUNK)]
        for i in range(NT):
            nc.sync.dma_start(kmn_sb[i][:], kmn_d[i])
            nc.sync.dma_start(xn_sb[i][:], xn_d[i])
        for i in range(1, NCHUNK):
            nc.sync.dma_start(g1t_sb[i][:], g1t_d[i])
        for i in range(2, NCHUNK):
            nc.sync.dma_start(g2t_sb[i][:], g2t_d[i])

        for h in range(HPC):
            gh = gcol_sb[:, h:h + 1]
            g1h = g1col_sb[:, h:h + 1]

            qrt_sb = [qrtp.tile([128, T], bf, tag="qrt", name="qrt_sb") for _ in range(2)]
            for i in range(2):
                nc.sync.dma_start(qrt_sb[i][:], qrt_d[h, i])
            v_sb = [vp.tile([128, D], bf, tag="v", name="v_sb") for _ in range(NT)]
            for i in range(NT):
                nc.sync.dma_start(v_sb[i][:], v_d[h, i])
            m0b_sb = [m0p.tile([128, D], bf, tag="m0b", name="m0b_sb") for _ in range(2)]
            for i in range(2):
                nc.sync.dma_start(m0b_sb[i][:], m0_d[h, i])

            # delta state: M in PSUM fp32, lagged bf16 copies + U history in SBUF
            m_ps = [mps.tile([128, D], f32, tag="mps", name="m_ps") for _ in range(2)]
            for i in range(2):
                nc.tensor.matmul(m_ps[i][:], ident_sb[:], m0b_sb[i][:],
                                 start=True, stop=False)
            msb = {0: m0b_sb}     # msb[c] = bf16 state before chunk c (lagged)
            usb = {}              # usb[c] = bf16 U_c

            ym_done = False
            sum_st = wk.tile([128, NT], f32, tag="sum_st")
            sq_st = wk.tile([128, NT], f32, tag="sq_st")
            gal_st = wk.tile([128, NT], f32, tag="gal_st")
            ngm_st = wk.tile([128, NT], f32, tag="ngm_st")
            ymg = [ymgp.tile([128, D], bf, tag="ymg", name="ymg_sb") for _ in range(NT)]

            for G in range(4):
                t0 = G * 512
                # ---- scoresT generation for this G window
                sT = {}
                for J in range(4 * G + 4):
                    sc_ps = mmps.tile([128, 512], f32, tag="mm", name="sc_ps")
                    nc.tensor.matmul(sc_ps[:], qrt_sb[0][:, J * 128:(J + 1) * 128],
                                     qrt_sb[0][:, t0:t0 + 512], start=True, stop=False)
                    nc.tensor.matmul(sc_ps[:], qrt_sb[1][:, J * 128:(J + 1) * 128],
                                     qrt_sb[1][:, t0:t0 + 512], start=False, stop=True)
                    st_sb = stp.tile([128, 512], bf, tag="st", name="st_sb")
                    off = J * 128 - t0
                    if off >= 0:
                        if off > 0:
                            nc.gpsimd.memset(st_sb[:, 0:off], 0.0)
                        nc.vector.scalar_tensor_tensor(
                            out=st_sb[:, off:off + 128], in0=sc_ps[:, off:off + 128],
                            scalar=g1h, in1=masku_sb[:], op0=ALU.mult, op1=ALU.mult)
                        if off + 128 < 512:
                            nc.vector.tensor_scalar_mul(
                                st_sb[:, off + 128:512], sc_ps[:, off + 128:512], g1h)
                    else:
                        if J % 2 == 0:
                            nc.vector.tensor_scalar_mul(st_sb[:], sc_ps[:], g1h)
                        else:
                            nc.scalar.activation(st_sb[:], sc_ps[:], AF.Identity,
                                                 scale=g1h, bias=zero_sb[:])
                    sT[J] = st_sb

                # ---- y_memory phase, emitted after G0 scores so PE has queued work
                if not ym_done:
                    ym_done = True
                    ym32 = [ymgp.tile([128, D], bf, tag="ym32", name="ym32_sb")
                            for _ in range(NT)]
                    for tt in range(NT):
                        ym_ps = mmps.tile([128, D], f32, tag="mm", name="ym_ps")
                        nc.tensor.matmul(ym_ps[:], qmt_sb[0][:, tt * 128:(tt + 1) * 128],
                                         m0b_sb[0][:], start=True, stop=False)
                        nc.tensor.matmul(ym_ps[:], qmt_sb[1][:, tt * 128:(tt + 1) * 128],
                                         m0b_sb[1][:], start=False, stop=True)
                        scr = wk.tile([128, D], bf, tag="scr")
                        nc.scalar.activation(scr[:], ym_ps[:], AF.Square,
                                             scale=float(1.0 / math.sqrt(D)),
                                             accum_out=sq_st[:, tt:tt + 1])
                        nc.scalar.activation(ym32[tt][:], ym_ps[:], AF.Copy,
                                             accum_out=sum_st[:, tt:tt + 1])
                    mu_st = wk.tile([128, NT], f32, tag="mu_st")
                    nc.vector.tensor_scalar_mul(mu_st[:], sum_st[:], float(1.0 / D))
                    mu2_st = wk.tile([128, NT], f32, tag="mu2_st")
                    nc.vector.tensor_tensor(out=mu2_st[:], in0=mu_st[:], in1=mu_st[:], op=ALU.mult)
                    var_st = wk.tile([128, NT], f32, tag="var_st")
                    nc.vector.tensor_tensor(out=var_st[:], in0=sq_st[:], in1=mu2_st[:], op=ALU.subtract)
                    nc.vector.tensor_scalar_add(var_st[:], var_st[:], float(LN_EPS))
                    sd_st = wk.tile([128, NT], f32, tag="sd_st")
                    nc.scalar.activation(sd_st[:], var_st[:], AF.Sqrt)
                    al_st = wk.tile([128, NT], f32, tag="al_st")
                    nc.vector.reciprocal(al_st[:], sd_st[:])
                    nc.vector.tensor_scalar_mul(gal_st[:], al_st[:], gh)
                    nc.vector.scalar_tensor_tensor(out=ngm_st[:], in0=mu_st[:], scalar=-1.0,
                                                   in1=gal_st[:], op0=ALU.mult, op1=ALU.mult)
                    for tt in range(NT):
                        nc.scalar.activation(ymg[tt][:], ym32[tt][:], AF.Identity,
                                             scale=gal_st[:, tt:tt + 1],
                                             bias=ngm_st[:, tt:tt + 1])

                # ---- y accumulation + one (lagged) delta chunk per t-tile
                for I in range(4 * G, 4 * G + 4):
                    y_ps = yps.tile([128, D], f32, tag="yps", name="y_ps")
                    off = I * 128 - t0
                    for J in range(I + 1):
                        nc.tensor.matmul(y_ps[:], sT[J][:, off:off + 128], v_sb[J][:],
                                         start=(J == 0), stop=(J == I))
                    y_out = outp.tile([128, D], f32, tag="y_out")
                    nc.vector.tensor_tensor(out=y_out[:], in0=y_ps[:],
                                            in1=ymg[I][:], op=ALU.add)
                    nc.sync.dma_start(y_d[h, I], y_out[:])

                    cc = I
                    base = max(cc - 2, 0)
                    at_sb = atp.tile([128, 128], bf, tag="at")
                    nc.sync.dma_start(at_sb[:], at_d[h, cc])
                    r_ps = mmps.tile([128, D], f32, tag="mm", name="r_ps")
                    mb = msb[base]
                    has_g1 = cc - 1 >= base
                    has_g2 = cc - 2 >= base
                    nc.tensor.matmul(r_ps[:], kmt_sb[0][:, cc * 128:(cc + 1) * 128],
                                     mb[0][:], start=True, stop=False)
                    nc.tensor.matmul(r_ps[:], kmt_sb[1][:, cc * 128:(cc + 1) * 128],
                                     mb[1][:], start=False,
                                     stop=not (has_g1 or has_g2))
                    if has_g2:
                        nc.tensor.matmul(r_ps[:], g2t_sb[cc][:], usb[cc - 2][:],
                                         start=False, stop=not has_g1)
                    if has_g1:
                        nc.tensor.matmul(r_ps[:], g1t_sb[cc][:], usb[cc - 1][:],
                                         start=False, stop=True)
                    u_in = wk.tile([128, D], bf, tag="u_in")
                    nc.vector.tensor_tensor(out=u_in[:], in0=xn_sb[cc][:],
                                            in1=r_ps[:], op=ALU.subtract)
                    u_ps = mmps.tile([128, D], f32, tag="mm", name="u_ps")
                    nc.tensor.matmul(u_ps[:], at_sb[:], u_in[:], start=True, stop=True)
                    u_sb = dl.tile([128, D], bf, tag="u_sb", name="u_sb")
                    nc.vector.tensor_copy(u_sb[:], u_ps[:])
                    usb[cc] = u_sb
                    last = (cc == NCHUNK - 1)
                    for i in range(2):
                        nc.tensor.matmul(m_ps[i][:],
                                         kmn_sb[cc][:, i * 128:(i + 1) * 128],
                                         u_sb[:], start=False, stop=last)
                    # lagged state copy (used 2 chunks later; off critical path)
                    if cc + 1 <= NCHUNK - 3:
                        nmsb = [dl.tile([128, D], bf, tag=f"msb{i}", name=f"msb{i}")
                                for i in range(2)]
                        nc.scalar.activation(nmsb[0][:], m_ps[0][:], AF.Copy)
                        nc.vector.tensor_copy(nmsb[1][:], m_ps[1][:])
                        msb[cc + 1] = nmsb

            # ---- M_new evacuation
            for i in range(2):
                mn_sb = outp.tile([128, D], f32, tag="mn")
                nc.vector.tensor_copy(mn_sb[:], m_ps[i][:])
                nc.sync.dma_start(mnew_d[h, i], mn_sb[:])

    nc.compile()
    return nc


def _get_program():
    global _PROGRAM
    if _PROGRAM is None:
        _PROGRAM = build_program()
    return _PROGRAM


# --------------------------------------------------------------------------
# public entry
# --------------------------------------------------------------------------

def _run(inputs, trace=False):
    nc = _get_program()
    in_maps = host_prep(inputs)
    kw = dict(trace=True, trace_cores=[0]) if trace else {}
    res = run_bass_kernel_spmd(nc, in_maps, list(range(NCORES)), **kw)
    y = np.zeros((B, NH, T, D), np.float32)
    M_new = np.zeros((B, NH, N, D), np.float32)
    for c in range(NCORES):
        b = c // 2
        h0 = (c % 2) * HPC
        y[b, h0:h0 + HPC] = res.results[c]["y"].reshape(HPC, T, D)
        M_new[b, h0:h0 + HPC] = res.results[c]["mnew"].reshape(HPC, N, D)
    return y, M_new, res.exec_time_ns


def kernel(**inputs):
    y, M_new, _ = _run(inputs, trace=False)
    return y, M_new


def run_profiled(inputs):
    return _run(inputs, trace=True)


# revision 6
# speedup vs baseline: 1.4592x; 1.0842x over previous
"""Trainium2 Bass kernel for nn_Attention_4612794875918.

Full inputs in, full outputs out. Internally shards across 8 NeuronCores:
core c handles batch b = c//2, head group hg = c%2 (4 heads each) — scores,
y_standard, memory matrix M and the delta scan are independent per (B, nh).

Device does all O(T^2) / O(T*N*D) matmul work per (b,h):
  - scoresT = rope(Q) @ rope(Q)^T (strictly-causal, computed transposed,
    staged bf16 in SBUF), y_std = scoresT^T @ V accumulated in PSUM
  - y_mem = Qm @ M0 with per-row LayerNorm fused via ACT scale/bias
  - delta-rule scan in 16 chunks of 128 with a 2-chunk-lagged state copy:
    R_c = K_c M_(c-2) + G2_c U_(c-2) + G1_c U_(c-1);  U_c = A_c (V_c - R_c);
    M += K_c^T U_c, M resident in PSUM fp32 across the scan. The lag keeps
    the PSUM->SBUF state copy off the PE critical path.
Host prep (cheap, O(T*N) / O(T*D)): RoPE tables + rotation of Q, the two
l2-normalized projections, beta/sigmoid, per-chunk triangular solve operators
A = (I + diag(b) tril(K K^T,-1))^{-1} diag(b), the chunk-coupling Gram
matrices G1/G2, layout transposes, bf16 casts.
"""
import math
import sys

import numpy as np
from ml_dtypes import bfloat16

if "/opt/trn_rl_repo" not in sys.path:
    sys.path.insert(0, "/opt/trn_rl_repo")

from contextlib import ExitStack

from concourse import bacc, mybir, tile  # noqa: E402
from concourse.bass_utils import run_bass_kernel_spmd  # noqa: E402

dt = mybir.dt
AF = mybir.ActivationFunctionType
ALU = mybir.AluOpType
AXL = mybir.AxisListType

B, NH, T, N, D = 4, 8, 2048, 256, 512
THETA = 2 ** 16
TWO_PI = 2.0 * math.pi
LN_EPS = 1e-5
C = 128              # delta chunk
NCHUNK = T // C      # 16
NT = T // 128        # 16 t-tiles
HPC = 4              # heads per core
NCORES = 8

_PROGRAM = None      # compile once per process


# --------------------------------------------------------------------------
# host prep
# --------------------------------------------------------------------------

def host_prep(inputs):
    Q = np.asarray(inputs["Q"], np.float32)
    V = np.asarray(inputs["V"], np.float32)
    x_raw = np.asarray(inputs["x_raw"], np.float32)
    x_next = np.asarray(inputs["x_next"], np.float32)
    Wq = np.asarray(inputs["theta_Q_w"], np.float32)
    Wk = np.asarray(inputs["theta_K_w"], np.float32)
    bw = np.asarray(inputs["beta_w"], np.float32)
    mg = np.asarray(inputs["memory_gate"], np.float32)
    M0 = np.asarray(inputs["M0"], np.float32)

    # rope -> QRT bf16 [B,NH,N,T]
    i = np.arange(N, dtype=np.float32)
    q = np.floor(i / 2.0) * 2.0
    freqs = (1.0 / (THETA ** (q / N)) / TWO_PI)
    ph = np.mod(np.arange(T, dtype=np.float32)[:, None] * freqs[None, :], 1.0) * TWO_PI
    pc, ps = np.cos(ph).astype(np.float32), np.sin(ph).astype(np.float32)
    Qe, Qo = Q[..., ::2], Q[..., 1::2]
    Qrot = np.empty_like(Q)
    Qrot[..., ::2] = -Qo
    Qrot[..., 1::2] = Qe
    QR = Q * pc + Qrot * ps
    QRT = np.ascontiguousarray(np.swapaxes(QR, -1, -2)).astype(bfloat16)

    Qm = x_raw @ Wq.T
    Qm /= np.maximum(np.linalg.norm(Qm, axis=-1, keepdims=True), 1e-12)
    Km = x_raw @ Wk.T
    Km /= np.maximum(np.linalg.norm(Km, axis=-1, keepdims=True), 1e-12)
    QmT = np.ascontiguousarray(np.swapaxes(Qm, -1, -2)).astype(bfloat16)  # [B,N,T]
    KmT = np.ascontiguousarray(np.swapaxes(Km, -1, -2)).astype(bfloat16)
    Kmn = Km.astype(bfloat16)                                            # [B,T,N]

    beta = 1.0 / (1.0 + np.exp(-(x_raw @ bw.T)))                         # [B,T,NH]
    KmC = Km.reshape(B, NCHUNK, C, N)
    S = np.einsum("bcik,bcjk->bcij", KmC, KmC)
    S_L = np.tril(S, -1)
    Ieye = np.eye(C, dtype=np.float32)
    bC = beta.reshape(B, NCHUNK, C, NH).transpose(0, 3, 1, 2)            # [B,NH,NCHUNK,C]
    Mats = Ieye[None, None, None] + bC[..., None] * S_L[:, None]
    A = np.linalg.inv(Mats) * bC[:, :, :, None, :]                       # [B,NH,NCHUNK,C,C]
    AT = np.ascontiguousarray(np.swapaxes(A, -1, -2)).astype(bfloat16)

    # chunk-coupling Gram matrices in lhsT form:
    # g1t[c] = Km_(c-1) Km_c^T (c>=1), g2t[c] = Km_(c-2) Km_c^T (c>=2)
    g1t = np.zeros((B, NCHUNK, C, C), np.float32)
    g2t = np.zeros((B, NCHUNK, C, C), np.float32)
    for b in range(B):
        for c in range(1, NCHUNK):
            g1t[b, c] = KmC[b, c - 1] @ KmC[b, c].T
        for c in range(2, NCHUNK):
            g2t[b, c] = KmC[b, c - 2] @ KmC[b, c].T
    g1t = g1t.astype(bfloat16)
    g2t = g2t.astype(bfloat16)

    g = (1.0 / (1.0 + np.exp(-mg.reshape(NH)))).astype(np.float32)

    masku = np.triu(np.ones((128, 128), np.float32), 1)
    ident = np.eye(128, dtype=np.float32).astype(bfloat16)

    in_maps = []
    for c in range(NCORES):
        b = c // 2
        h0 = (c % 2) * HPC
        gloc = g[h0:h0 + HPC]
        in_maps.append(dict(
            qrt=np.ascontiguousarray(QRT[b, h0:h0 + HPC]).reshape(HPC, 2, 128, T),
            qmt=np.ascontiguousarray(QmT[b]).reshape(2, 128, T),
            kmt=np.ascontiguousarray(KmT[b]).reshape(2, 128, T),
            kmn=np.ascontiguousarray(Kmn[b]).reshape(NT, 128, N),
            at=np.ascontiguousarray(AT[b, h0:h0 + HPC]),                  # [4,16,128,128] bf16
            g1t=g1t[b], g2t=g2t[b],                                       # [16,128,128] bf16
            v=np.ascontiguousarray(V[b, h0:h0 + HPC].astype(bfloat16)).reshape(HPC, NT, 128, D),
            xn=np.ascontiguousarray(x_next[b]).reshape(NT, 128, D),
            m0=np.ascontiguousarray(M0[b, h0:h0 + HPC].astype(bfloat16)).reshape(HPC, 2, 128, D),
            masku=masku,
            ident=ident,
            gcol=np.broadcast_to(gloc, (128, HPC)).copy(),
            g1col=np.broadcast_to(1.0 - gloc, (128, HPC)).copy(),
        ))
    return in_maps


# --------------------------------------------------------------------------
# device program (identical on all cores)
# --------------------------------------------------------------------------

def build_program():
    nc = bacc.Bacc("TRN2", target_bir_lowering=False, debug=False,
                   num_devices=NCORES)
    bf = dt.bfloat16
    f32 = dt.float32

    qrt_d = nc.dram_tensor("qrt", [HPC, 2, 128, T], bf, kind="ExternalInput").ap()
    qmt_d = nc.dram_tensor("qmt", [2, 128, T], bf, kind="ExternalInput").ap()
    kmt_d = nc.dram_tensor("kmt", [2, 128, T], bf, kind="ExternalInput").ap()
    kmn_d = nc.dram_tensor("kmn", [NT, 128, N], bf, kind="ExternalInput").ap()
    at_d = nc.dram_tensor("at", [HPC, NCHUNK, 128, 128], bf, kind="ExternalInput").ap()
    g1t_d = nc.dram_tensor("g1t", [NCHUNK, 128, 128], bf, kind="ExternalInput").ap()
    g2t_d = nc.dram_tensor("g2t", [NCHUNK, 128, 128], bf, kind="ExternalInput").ap()
    v_d = nc.dram_tensor("v", [HPC, NT, 128, D], bf, kind="ExternalInput").ap()
    xn_d = nc.dram_tensor("xn", [NT, 128, D], f32, kind="ExternalInput").ap()
    m0_d = nc.dram_tensor("m0", [HPC, 2, 128, D], bf, kind="ExternalInput").ap()
    masku_d = nc.dram_tensor("masku", [128, 128], f32, kind="ExternalInput").ap()
    ident_d = nc.dram_tensor("ident", [128, 128], bf, kind="ExternalInput").ap()
    gcol_d = nc.dram_tensor("gcol", [128, HPC], f32, kind="ExternalInput").ap()
    g1col_d = nc.dram_tensor("g1col", [128, HPC], f32, kind="ExternalInput").ap()
    y_d = nc.dram_tensor("y", [HPC, NT, 128, D], f32, kind="ExternalOutput").ap()
    mnew_d = nc.dram_tensor("mnew", [HPC, 2, 128, D], f32, kind="ExternalOutput").ap()

    with tile.TileContext(nc) as tc, ExitStack() as ctx:
        cst = ctx.enter_context(tc.tile_pool(name="cst", bufs=1))
        qrtp = ctx.enter_context(tc.tile_pool(name="qrtp", bufs=3))
        vp = ctx.enter_context(tc.tile_pool(name="vp", bufs=18))
        stp = ctx.enter_context(tc.tile_pool(name="stp", bufs=18))
        ymgp = ctx.enter_context(tc.tile_pool(name="ymgp", bufs=16))
        wk = ctx.enter_context(tc.tile_pool(name="wk", bufs=2))
        dl = ctx.enter_context(tc.tile_pool(name="dl", bufs=4))      # delta lagged state
        m0p = ctx.enter_context(tc.tile_pool(name="m0p", bufs=4))
        atp = ctx.enter_context(tc.tile_pool(name="atp", bufs=3))
        outp = ctx.enter_context(tc.tile_pool(name="outp", bufs=3))
        mmps = ctx.enter_context(tc.tile_pool(name="mmps", bufs=4, space="PSUM"))
        yps = ctx.enter_context(tc.tile_pool(name="yps", bufs=2, space="PSUM"))
        mps = ctx.enter_context(tc.tile_pool(name="mps", bufs=2, space="PSUM"))

        # ---- persistent constants / per-core tensors
        masku_sb = cst.tile([128, 128], f32, tag="masku")
        ident_sb = cst.tile([128, 128], bf, tag="ident")
        gcol_sb = cst.tile([128, HPC], f32, tag="gcol")
        g1col_sb = cst.tile([128, HPC], f32, tag="g1col")
        zero_sb = cst.tile([128, 1], f32, tag="zero")
        nc.sync.dma_start(masku_sb[:], masku_d[:])
        nc.sync.dma_start(ident_sb[:], ident_d[:])
        nc.sync.dma_start(gcol_sb[:], gcol_d[:])
        nc.sync.dma_start(g1col_sb[:], g1col_d[:])
        nc.gpsimd.memset(zero_sb[:], 0.0)

        qmt_sb = [cst.tile([128, T], bf, tag=f"qmt{i}", name=f"qmt_sb{i}") for i in range(2)]
        kmt_sb = [cst.tile([128, T], bf, tag=f"kmt{i}", name=f"kmt_sb{i}") for i in range(2)]
        for i in range(2):
            nc.sync.dma_start(qmt_sb[i][:], qmt_d[i])
            nc.sync.dma_start(kmt_sb[i][:], kmt_d[i])
        kmn_sb = [cst.tile([128, N], bf, tag=f"kmn{i}", name=f"kmn_sb{i}") for i in range(NT)]
        xn_sb = [cst.tile([128, D], f32, tag=f"xn{i}", name=f"xn_sb{i}") for i in range(NT)]
        g1t_sb = [cst.tile([128, 128], bf, tag=f"g1t{i}", name=f"g1t_sb{i}") for i in range(NCHUNK)]
        g2t_sb = [cst.tile([128, 128], bf, tag=f"g2t{i}", name=f"g2t_sb{i}") for i in range(NCHUNK)]

        for h in range(HPC):
            gh = gcol_sb[:, h:h + 1]
            g1h = g1col_sb[:, h:h + 1]

            qrt_sb = [qrtp.tile([128, T], bf, tag="qrt", name="qrt_sb") for _ in range(2)]
            for i in range(2):
                nc.sync.dma_start(qrt_sb[i][:], qrt_d[h, i])
            v_sb = [vp.tile([128, D], bf, tag="v", name="v_sb") for _ in range(NT)]
            for i in range(NT):
                nc.sync.dma_start(v_sb[i][:], v_d[h, i])
            m0b_sb = [m0p.tile([128, D], bf, tag="m0b", name="m0b_sb") for _ in range(2)]
            for i in range(2):
                nc.sync.dma_start(m0b_sb[i][:], m0_d[h, i])
            if h == 0:
                # heavy shared loads deferred behind pair-0 working set
                for i in range(NT):
                    nc.sync.dma_start(kmn_sb[i][:], kmn_d[i])
                    nc.sync.dma_start(xn_sb[i][:], xn_d[i])
                for i in range(1, NCHUNK):
                    nc.sync.dma_start(g1t_sb[i][:], g1t_d[i])
                for i in range(2, NCHUNK):
                    nc.sync.dma_start(g2t_sb[i][:], g2t_d[i])

            # delta state: M in PSUM fp32, lagged bf16 copies + U history in SBUF
            m_ps = [mps.tile([128, D], f32, tag="mps", name="m_ps") for _ in range(2)]
            for i in range(2):
                nc.tensor.matmul(m_ps[i][:], ident_sb[:], m0b_sb[i][:],
                                 start=True, stop=False)
            msb = {0: m0b_sb}     # msb[c] = bf16 state before chunk c (lagged)
            usb = {}              # usb[c] = bf16 U_c

            ym_done = False
            sum_st = wk.tile([128, NT], f32, tag="sum_st")
            sq_st = wk.tile([128, NT], f32, tag="sq_st")
            gal_st = wk.tile([128, NT], f32, tag="gal_st")
            ngm_st = wk.tile([128, NT], f32, tag="ngm_st")
            ymg = [ymgp.tile([128, D], bf, tag="ymg", name="ymg_sb") for _ in range(NT)]

            for G in range(4):
                t0 = G * 512
                # ---- scoresT generation for this G window
                sT = {}
                for J in range(4 * G + 4):
                    sc_ps = mmps.tile([128, 512], f32, tag="mm", name="sc_ps")
                    nc.tensor.matmul(sc_ps[:], qrt_sb[0][:, J * 128:(J + 1) * 128],
                                     qrt_sb[0][:, t0:t0 + 512], start=True, stop=False)
                    nc.tensor.matmul(sc_ps[:], qrt_sb[1][:, J * 128:(J + 1) * 128],
                                     qrt_sb[1][:, t0:t0 + 512], start=False, stop=True)
                    st_sb = stp.tile([128, 512], bf, tag="st", name="st_sb")
                    off = J * 128 - t0
                    if off >= 0:
                        if off > 0:
                            nc.gpsimd.memset(st_sb[:, 0:off], 0.0)
                        nc.vector.scalar_tensor_tensor(
                            out=st_sb[:, off:off + 128], in0=sc_ps[:, off:off + 128],
                            scalar=g1h, in1=masku_sb[:], op0=ALU.mult, op1=ALU.mult)
                        if off + 128 < 512:
                            nc.vector.tensor_scalar_mul(
                                st_sb[:, off + 128:512], sc_ps[:, off + 128:512], g1h)
                    else:
                        if J % 2 == 0:
                            nc.vector.tensor_scalar_mul(st_sb[:], sc_ps[:], g1h)
                        else:
                            nc.scalar.activation(st_sb[:], sc_ps[:], AF.Identity,
                                                 scale=g1h, bias=zero_sb[:])
                    sT[J] = st_sb

                # ---- y_memory for this G's 4 t-tiles (spread across groups)
                ym32_g = []
                for tt in range(4 * G, 4 * G + 4):
                    ym_ps = mmps.tile([128, D], f32, tag="mm", name="ym_ps")
                    nc.tensor.matmul(ym_ps[:], qmt_sb[0][:, tt * 128:(tt + 1) * 128],
                                     m0b_sb[0][:], start=True, stop=False)
                    nc.tensor.matmul(ym_ps[:], qmt_sb[1][:, tt * 128:(tt + 1) * 128],
                                     m0b_sb[1][:], start=False, stop=True)
                    scr = wk.tile([128, D], bf, tag="scr")
                    nc.scalar.activation(scr[:], ym_ps[:], AF.Square,
                                         scale=float(1.0 / math.sqrt(D)),
                                         accum_out=sq_st[:, tt:tt + 1])
                    ym32 = ymgp.tile([128, D], bf, tag="ym32", name="ym32_sb")
                    nc.vector.tensor_copy(ym32[:], ym_ps[:])
                    nc.vector.tensor_reduce(out=sum_st[:, tt:tt + 1], in_=ym32[:],
                                            axis=AXL.X, op=ALU.add)
                    ym32_g.append(ym32)
                gsl = slice(4 * G, 4 * G + 4)
                mu_st = wk.tile([128, 4], f32, tag="mu_st")
                nc.vector.tensor_scalar_mul(mu_st[:], sum_st[:, gsl], float(1.0 / D))
                mu2_st = wk.tile([128, 4], f32, tag="mu2_st")
                nc.vector.tensor_tensor(out=mu2_st[:], in0=mu_st[:], in1=mu_st[:], op=ALU.mult)
                var_st = wk.tile([128, 4], f32, tag="var_st")
                nc.vector.tensor_tensor(out=var_st[:], in0=sq_st[:, gsl], in1=mu2_st[:], op=ALU.subtract)
                nc.vector.tensor_scalar_add(var_st[:], var_st[:], float(LN_EPS))
                sd_st = wk.tile([128, 4], f32, tag="sd_st")
                nc.scalar.activation(sd_st[:], var_st[:], AF.Sqrt)
                al_st = wk.tile([128, 4], f32, tag="al_st")
                nc.vector.reciprocal(al_st[:], sd_st[:])
                nc.vector.tensor_scalar_mul(gal_st[:, gsl], al_st[:], gh)
                nc.vector.scalar_tensor_tensor(out=ngm_st[:, gsl], in0=mu_st[:], scalar=-1.0,
                                               in1=gal_st[:, gsl], op0=ALU.mult, op1=ALU.mult)
                for k, tt in enumerate(range(4 * G, 4 * G + 4)):
                    nc.scalar.activation(ymg[tt][:], ym32_g[k][:], AF.Identity,
                                         scale=gal_st[:, tt:tt + 1],
                                         bias=ngm_st[:, tt:tt + 1])

                # ---- y accumulation + one (lagged) delta chunk per t-tile
                for I in range(4 * G, 4 * G + 4):
                    y_ps = yps.tile([128, D], f32, tag="yps", name="y_ps")
                    off = I * 128 - t0
                    for J in range(I + 1):
                        nc.tensor.matmul(y_ps[:], sT[J][:, off:off + 128], v_sb[J][:],
                                         start=(J == 0), stop=(J == I))
                    y_out = outp.tile([128, D], f32, tag="y_out")
                    nc.vector.tensor_tensor(out=y_out[:], in0=y_ps[:],
                                            in1=ymg[I][:], op=ALU.add)
                    nc.sync.dma_start(y_d[h, I], y_out[:])

                    cc = I
                    base = max(cc - 2, 0)
                    at_sb = atp.tile([128, 128], bf, tag="at")
                    nc.sync.dma_start(at_sb[:], at_d[h, cc])
                    r_ps = mmps.tile([128, D], f32, tag="mm", name="r_ps")
                    mb = msb[base]
                    has_g1 = cc - 1 >= base
                    has_g2 = cc - 2 >= base
                    nc.tensor.matmul(r_ps[:], kmt_sb[0][:, cc * 128:(cc + 1) * 128],
                                     mb[0][:], start=True, stop=False)
                    nc.tensor.matmul(r_ps[:], kmt_sb[1][:, cc * 128:(cc + 1) * 128],
                                     mb[1][:], start=False,
                                     stop=not (has_g1 or has_g2))
                    if has_g2:
                        nc.tensor.matmul(r_ps[:], g2t_sb[cc][:], usb[cc - 2][:],
                                         start=False, stop=not has_g1)
                    if has_g1:
                        nc.tensor.matmul(r_ps[:], g1t_sb[cc][:], usb[cc - 1][:],
                                         start=False, stop=True)
                    u_in = wk.tile([128, D], bf, tag="u_in")
                    nc.vector.tensor_tensor(out=u_in[:], in0=xn_sb[cc][:],
                                            in1=r_ps[:], op=ALU.subtract)
                    u_ps = mmps.tile([128, D], f32, tag="mm", name="u_ps")
                    nc.tensor.matmul(u_ps[:], at_sb[:], u_in[:], start=True, stop=True)
                    u_sb = dl.tile([128, D], bf, tag="u_sb", name="u_sb")
                    nc.vector.tensor_copy(u_sb[:], u_ps[:])
                    usb[cc] = u_sb
                    last = (cc == NCHUNK - 1)
                    for i in range(2):
                        nc.tensor.matmul(m_ps[i][:],
                                         kmn_sb[cc][:, i * 128:(i + 1) * 128],
                                         u_sb[:], start=False, stop=last)
                    # lagged state copy (used 2 chunks later; off critical path)
                    if cc + 1 <= NCHUNK - 3:
                        nmsb = [dl.tile([128, D], bf, tag=f"msb{i}", name=f"msb{i}")
                                for i in range(2)]
                        nc.scalar.activation(nmsb[0][:], m_ps[0][:], AF.Copy)
                        nc.vector.tensor_copy(nmsb[1][:], m_ps[1][:])
                        msb[cc + 1] = nmsb

            # ---- M_new evacuation
            for i in range(2):
                mn_sb = outp.tile([128, D], f32, tag="mn")
                nc.vector.tensor_copy(mn_sb[:], m_ps[i][:])
                nc.sync.dma_start(mnew_d[h, i], mn_sb[:])

    nc.compile()
    return nc


def _get_program():
    global _PROGRAM
    if _PROGRAM is None:
        _PROGRAM = build_program()
    return _PROGRAM


# --------------------------------------------------------------------------
# public entry
# --------------------------------------------------------------------------

def _run(inputs, trace=False):
    nc = _get_program()
    in_maps = host_prep(inputs)
    kw = dict(trace=True, trace_cores=[0]) if trace else {}
    res = run_bass_kernel_spmd(nc, in_maps, list(range(NCORES)), **kw)
    y = np.zeros((B, NH, T, D), np.float32)
    M_new = np.zeros((B, NH, N, D), np.float32)
    for c in range(NCORES):
        b = c // 2
        h0 = (c % 2) * HPC
        y[b, h0:h0 + HPC] = res.results[c]["y"].reshape(HPC, T, D)
        M_new[b, h0:h0 + HPC] = res.results[c]["mnew"].reshape(HPC, N, D)
    return y, M_new, res.exec_time_ns


def kernel(**inputs):
    y, M_new, _ = _run(inputs, trace=False)
    return y, M_new


def run_profiled(inputs):
    return _run(inputs, trace=True)


# revision 7
# speedup vs baseline: 1.5290x; 1.0478x over previous
"""Trainium2 Bass kernel for nn_Attention_4612794875918.

Full inputs in, full outputs out. Internally shards across 8 NeuronCores:
core c handles batch b = c//2, head group hg = c%2 (4 heads each) — scores,
y_standard, memory matrix M and the delta scan are independent per (B, nh).

Device does all O(T^2) / O(T*N*D) matmul work per (b,h):
  - scoresT = rope(Q) @ rope(Q)^T (strictly-causal, computed transposed,
    staged bf16 in SBUF), y_std = scoresT^T @ V accumulated in PSUM
  - y_mem = Qm @ M0 with per-row LayerNorm fused via ACT scale/bias
  - delta-rule scan in 16 chunks of 128 with a 2-chunk-lagged state copy:
    R_c = K_c M_(c-2) + G2_c U_(c-2) + G1_c U_(c-1);  U_c = A_c (V_c - R_c);
    M += K_c^T U_c, M resident in PSUM fp32 across the scan. The lag keeps
    the PSUM->SBUF state copy off the PE critical path.
Host prep (cheap, O(T*N) / O(T*D)): RoPE tables + rotation of Q, the two
l2-normalized projections, beta/sigmoid, per-chunk triangular solve operators
A = (I + diag(b) tril(K K^T,-1))^{-1} diag(b), the chunk-coupling Gram
matrices G1/G2, layout transposes, bf16 casts.
"""
import math
import sys

import numpy as np
from ml_dtypes import bfloat16

if "/opt/trn_rl_repo" not in sys.path:
    sys.path.insert(0, "/opt/trn_rl_repo")

from contextlib import ExitStack

from concourse import bacc, mybir, tile  # noqa: E402
from concourse.bass_utils import run_bass_kernel_spmd  # noqa: E402

dt = mybir.dt
AF = mybir.ActivationFunctionType
ALU = mybir.AluOpType
AXL = mybir.AxisListType

B, NH, T, N, D = 4, 8, 2048, 256, 512
THETA = 2 ** 16
TWO_PI = 2.0 * math.pi
LN_EPS = 1e-5
C = 128              # delta chunk
NCHUNK = T // C      # 16
NT = T // 128        # 16 t-tiles
HPC = 4              # heads per core
NCORES = 8

_PROGRAM = None      # compile once per process


# --------------------------------------------------------------------------
# host prep
# --------------------------------------------------------------------------

def host_prep(inputs):
    Q = np.asarray(inputs["Q"], np.float32)
    V = np.asarray(inputs["V"], np.float32)
    x_raw = np.asarray(inputs["x_raw"], np.float32)
    x_next = np.asarray(inputs["x_next"], np.float32)
    Wq = np.asarray(inputs["theta_Q_w"], np.float32)
    Wk = np.asarray(inputs["theta_K_w"], np.float32)
    bw = np.asarray(inputs["beta_w"], np.float32)
    mg = np.asarray(inputs["memory_gate"], np.float32)
    M0 = np.asarray(inputs["M0"], np.float32)

    # rope -> QRT bf16 [B,NH,N,T]
    i = np.arange(N, dtype=np.float32)
    q = np.floor(i / 2.0) * 2.0
    freqs = (1.0 / (THETA ** (q / N)) / TWO_PI)
    ph = np.mod(np.arange(T, dtype=np.float32)[:, None] * freqs[None, :], 1.0) * TWO_PI
    pc, ps = np.cos(ph).astype(np.float32), np.sin(ph).astype(np.float32)
    Qe, Qo = Q[..., ::2], Q[..., 1::2]
    Qrot = np.empty_like(Q)
    Qrot[..., ::2] = -Qo
    Qrot[..., 1::2] = Qe
    QR = Q * pc + Qrot * ps
    QRT = np.ascontiguousarray(np.swapaxes(QR, -1, -2)).astype(bfloat16)

    Qm = x_raw @ Wq.T
    Qm /= np.maximum(np.linalg.norm(Qm, axis=-1, keepdims=True), 1e-12)
    Km = x_raw @ Wk.T
    Km /= np.maximum(np.linalg.norm(Km, axis=-1, keepdims=True), 1e-12)
    QmT = np.ascontiguousarray(np.swapaxes(Qm, -1, -2)).astype(bfloat16)  # [B,N,T]
    KmT = np.ascontiguousarray(np.swapaxes(Km, -1, -2)).astype(bfloat16)
    Kmn = Km.astype(bfloat16)                                            # [B,T,N]

    beta = 1.0 / (1.0 + np.exp(-(x_raw @ bw.T)))                         # [B,T,NH]
    KmC = Km.reshape(B, NCHUNK, C, N)
    S = np.einsum("bcik,bcjk->bcij", KmC, KmC)
    S_L = np.tril(S, -1)
    Ieye = np.eye(C, dtype=np.float32)
    bC = beta.reshape(B, NCHUNK, C, NH).transpose(0, 3, 1, 2)            # [B,NH,NCHUNK,C]
    Mats = Ieye[None, None, None] + bC[..., None] * S_L[:, None]
    A = np.linalg.inv(Mats) * bC[:, :, :, None, :]                       # [B,NH,NCHUNK,C,C]
    AT = np.ascontiguousarray(np.swapaxes(A, -1, -2)).astype(bfloat16)

    # chunk-coupling Gram matrices in lhsT form:
    # g1t[c] = Km_(c-1) Km_c^T (c>=1), g2t[c] = Km_(c-2) Km_c^T (c>=2)
    g1t = np.zeros((B, NCHUNK, C, C), np.float32)
    g2t = np.zeros((B, NCHUNK, C, C), np.float32)
    for b in range(B):
        for c in range(1, NCHUNK):
            g1t[b, c] = KmC[b, c - 1] @ KmC[b, c].T
        for c in range(2, NCHUNK):
            g2t[b, c] = KmC[b, c - 2] @ KmC[b, c].T
    g1t = g1t.astype(bfloat16)
    g2t = g2t.astype(bfloat16)

    g = (1.0 / (1.0 + np.exp(-mg.reshape(NH)))).astype(np.float32)

    masku = np.triu(np.ones((128, 128), np.float32), 1)
    ident = np.eye(128, dtype=np.float32).astype(bfloat16)

    in_maps = []
    for c in range(NCORES):
        b = c // 2
        h0 = (c % 2) * HPC
        gloc = g[h0:h0 + HPC]
        in_maps.append(dict(
            qrt=np.ascontiguousarray(QRT[b, h0:h0 + HPC]).reshape(HPC, 2, 128, T),
            qmt=np.ascontiguousarray(QmT[b]).reshape(2, 128, T),
            kmt=np.ascontiguousarray(KmT[b]).reshape(2, 128, T),
            kmn=np.ascontiguousarray(Kmn[b]).reshape(NT, 128, N),
            at=np.ascontiguousarray(AT[b, h0:h0 + HPC]),                  # [4,16,128,128] bf16
            g1t=g1t[b], g2t=g2t[b],                                       # [16,128,128] bf16
            v=np.ascontiguousarray(V[b, h0:h0 + HPC].astype(bfloat16)).reshape(HPC, NT, 128, D),
            xn=np.ascontiguousarray(x_next[b]).reshape(NT, 128, D),
            m0=np.ascontiguousarray(M0[b, h0:h0 + HPC].astype(bfloat16)).reshape(HPC, 2, 128, D),
            masku=masku,
            ident=ident,
            gcol=np.broadcast_to(gloc, (128, HPC)).copy(),
            g1col=np.broadcast_to(1.0 - gloc, (128, HPC)).copy(),
        ))
    return in_maps


# --------------------------------------------------------------------------
# device program (identical on all cores)
# --------------------------------------------------------------------------

def build_program():
    nc = bacc.Bacc("TRN2", target_bir_lowering=False, debug=False,
                   num_devices=NCORES)
    bf = dt.bfloat16
    f32 = dt.float32

    qrt_d = nc.dram_tensor("qrt", [HPC, 2, 128, T], bf, kind="ExternalInput").ap()
    qmt_d = nc.dram_tensor("qmt", [2, 128, T], bf, kind="ExternalInput").ap()
    kmt_d = nc.dram_tensor("kmt", [2, 128, T], bf, kind="ExternalInput").ap()
    kmn_d = nc.dram_tensor("kmn", [NT, 128, N], bf, kind="ExternalInput").ap()
    at_d = nc.dram_tensor("at", [HPC, NCHUNK, 128, 128], bf, kind="ExternalInput").ap()
    g1t_d = nc.dram_tensor("g1t", [NCHUNK, 128, 128], bf, kind="ExternalInput").ap()
    g2t_d = nc.dram_tensor("g2t", [NCHUNK, 128, 128], bf, kind="ExternalInput").ap()
    v_d = nc.dram_tensor("v", [HPC, NT, 128, D], bf, kind="ExternalInput").ap()
    xn_d = nc.dram_tensor("xn", [NT, 128, D], f32, kind="ExternalInput").ap()
    m0_d = nc.dram_tensor("m0", [HPC, 2, 128, D], bf, kind="ExternalInput").ap()
    masku_d = nc.dram_tensor("masku", [128, 128], f32, kind="ExternalInput").ap()
    ident_d = nc.dram_tensor("ident", [128, 128], bf, kind="ExternalInput").ap()
    gcol_d = nc.dram_tensor("gcol", [128, HPC], f32, kind="ExternalInput").ap()
    g1col_d = nc.dram_tensor("g1col", [128, HPC], f32, kind="ExternalInput").ap()
    y_d = nc.dram_tensor("y", [HPC, NT, 128, D], f32, kind="ExternalOutput").ap()
    mnew_d = nc.dram_tensor("mnew", [HPC, 2, 128, D], f32, kind="ExternalOutput").ap()

    with tile.TileContext(nc) as tc, ExitStack() as ctx:
        cst = ctx.enter_context(tc.tile_pool(name="cst", bufs=1))
        qrtp = ctx.enter_context(tc.tile_pool(name="qrtp", bufs=3))
        vp = ctx.enter_context(tc.tile_pool(name="vp", bufs=18))
        stp = ctx.enter_context(tc.tile_pool(name="stp", bufs=18))
        ymgp = ctx.enter_context(tc.tile_pool(name="ymgp", bufs=16))
        wk = ctx.enter_context(tc.tile_pool(name="wk", bufs=2))
        dl = ctx.enter_context(tc.tile_pool(name="dl", bufs=4))      # delta lagged state
        m0p = ctx.enter_context(tc.tile_pool(name="m0p", bufs=4))
        atp = ctx.enter_context(tc.tile_pool(name="atp", bufs=3))
        outp = ctx.enter_context(tc.tile_pool(name="outp", bufs=3))
        mmps = ctx.enter_context(tc.tile_pool(name="mmps", bufs=4, space="PSUM"))
        yps = ctx.enter_context(tc.tile_pool(name="yps", bufs=2, space="PSUM"))
        mps = ctx.enter_context(tc.tile_pool(name="mps", bufs=2, space="PSUM"))

        # ---- persistent constants / per-core tensors
        masku_sb = cst.tile([128, 128], f32, tag="masku")
        ident_sb = cst.tile([128, 128], bf, tag="ident")
        gcol_sb = cst.tile([128, HPC], f32, tag="gcol")
        g1col_sb = cst.tile([128, HPC], f32, tag="g1col")
        zero_sb = cst.tile([128, 1], f32, tag="zero")
        nc.sync.dma_start(masku_sb[:], masku_d[:])
        nc.sync.dma_start(ident_sb[:], ident_d[:])
        nc.sync.dma_start(gcol_sb[:], gcol_d[:])
        nc.sync.dma_start(g1col_sb[:], g1col_d[:])
        nc.gpsimd.memset(zero_sb[:], 0.0)

        qmt_sb = [cst.tile([128, T], bf, tag=f"qmt{i}", name=f"qmt_sb{i}") for i in range(2)]
        kmt_sb = [cst.tile([128, T], bf, tag=f"kmt{i}", name=f"kmt_sb{i}") for i in range(2)]
        for i in range(2):
            nc.sync.dma_start(qmt_sb[i][:], qmt_d[i])
            nc.sync.dma_start(kmt_sb[i][:], kmt_d[i])
        kmn_sb = [cst.tile([128, N], bf, tag=f"kmn{i}", name=f"kmn_sb{i}") for i in range(NT)]
        xn_sb = [cst.tile([128, D], f32, tag=f"xn{i}", name=f"xn_sb{i}") for i in range(NT)]
        g1t_sb = [cst.tile([128, 128], bf, tag=f"g1t{i}", name=f"g1t_sb{i}") for i in range(NCHUNK)]
        g2t_sb = [cst.tile([128, 128], bf, tag=f"g2t{i}", name=f"g2t_sb{i}") for i in range(NCHUNK)]

        for h in range(HPC):
            gh = gcol_sb[:, h:h + 1]
            g1h = g1col_sb[:, h:h + 1]

            qrt_sb = [qrtp.tile([128, T], bf, tag="qrt", name="qrt_sb") for _ in range(2)]
            for i in range(2):
                nc.sync.dma_start(qrt_sb[i][:], qrt_d[h, i])
            v_sb = [vp.tile([128, D], bf, tag="v", name="v_sb") for _ in range(NT)]
            for i in range(NT):
                nc.sync.dma_start(v_sb[i][:], v_d[h, i])
            m0b_sb = [m0p.tile([128, D], bf, tag="m0b", name="m0b_sb") for _ in range(2)]
            for i in range(2):
                nc.sync.dma_start(m0b_sb[i][:], m0_d[h, i])
            if h == 0:
                # heavy shared loads deferred behind pair-0 working set,
                # interleaved in delta-chunk order so chunk 0 unblocks first
                for i in range(NT):
                    nc.sync.dma_start(kmn_sb[i][:], kmn_d[i])
                    nc.sync.dma_start(xn_sb[i][:], xn_d[i])
                    if i >= 1:
                        nc.sync.dma_start(g1t_sb[i][:], g1t_d[i])
                    if i >= 2:
                        nc.sync.dma_start(g2t_sb[i][:], g2t_d[i])

            # delta state: M in PSUM fp32, lagged bf16 copies + U history in SBUF
            m_ps = [mps.tile([128, D], f32, tag="mps", name="m_ps") for _ in range(2)]
            for i in range(2):
                nc.tensor.matmul(m_ps[i][:], ident_sb[:], m0b_sb[i][:],
                                 start=True, stop=False)
            msb = {0: m0b_sb}     # msb[c] = bf16 state before chunk c (lagged)
            usb = {}              # usb[c] = bf16 U_c

            ym_done = False
            sum_st = wk.tile([128, NT], f32, tag="sum_st")
            sq_st = wk.tile([128, NT], f32, tag="sq_st")
            gal_st = wk.tile([128, NT], f32, tag="gal_st")
            ngm_st = wk.tile([128, NT], f32, tag="ngm_st")
            ymg = [ymgp.tile([128, D], bf, tag="ymg", name="ymg_sb") for _ in range(NT)]

            for G in range(4):
                t0 = G * 512
                # ---- scoresT generation for this G window
                sT = {}
                for J in range(4 * G + 4):
                    sc_ps = mmps.tile([128, 512], f32, tag="mm", name="sc_ps")
                    nc.tensor.matmul(sc_ps[:], qrt_sb[0][:, J * 128:(J + 1) * 128],
                                     qrt_sb[0][:, t0:t0 + 512], start=True, stop=False)
                    nc.tensor.matmul(sc_ps[:], qrt_sb[1][:, J * 128:(J + 1) * 128],
                                     qrt_sb[1][:, t0:t0 + 512], start=False, stop=True)
                    st_sb = stp.tile([128, 512], bf, tag="st", name="st_sb")
                    off = J * 128 - t0
                    if off >= 0:
                        if off > 0:
                            nc.gpsimd.memset(st_sb[:, 0:off], 0.0)
                        nc.vector.scalar_tensor_tensor(
                            out=st_sb[:, off:off + 128], in0=sc_ps[:, off:off + 128],
                            scalar=g1h, in1=masku_sb[:], op0=ALU.mult, op1=ALU.mult)
                        if off + 128 < 512:
                            nc.vector.tensor_scalar_mul(
                                st_sb[:, off + 128:512], sc_ps[:, off + 128:512], g1h)
                    else:
                        if J % 2 == 0:
                            nc.vector.tensor_scalar_mul(st_sb[:], sc_ps[:], g1h)
                        else:
                            nc.scalar.activation(st_sb[:], sc_ps[:], AF.Identity,
                                                 scale=g1h, bias=zero_sb[:])
                    sT[J] = st_sb

                # ---- y_memory for this G's 4 t-tiles (spread across groups)
                ym32_g = []
                for tt in range(4 * G, 4 * G + 4):
                    ym_ps = mmps.tile([128, D], f32, tag="mm", name="ym_ps")
                    nc.tensor.matmul(ym_ps[:], qmt_sb[0][:, tt * 128:(tt + 1) * 128],
                                     m0b_sb[0][:], start=True, stop=False)
                    nc.tensor.matmul(ym_ps[:], qmt_sb[1][:, tt * 128:(tt + 1) * 128],
                                     m0b_sb[1][:], start=False, stop=True)
                    scr = wk.tile([128, D], bf, tag="scr")
                    nc.scalar.activation(scr[:], ym_ps[:], AF.Square,
                                         scale=float(1.0 / math.sqrt(D)),
                                         accum_out=sq_st[:, tt:tt + 1])
                    ym32 = ymgp.tile([128, D], bf, tag="ym32", name="ym32_sb")
                    nc.scalar.activation(ym32[:], ym_ps[:], AF.Copy,
                                         accum_out=sum_st[:, tt:tt + 1])
                    ym32_g.append(ym32)
                gsl = slice(4 * G, 4 * G + 4)
                mu_st = wk.tile([128, 4], f32, tag="mu_st")
                nc.vector.tensor_scalar_mul(mu_st[:], sum_st[:, gsl], float(1.0 / D))
                mu2_st = wk.tile([128, 4], f32, tag="mu2_st")
                nc.vector.tensor_tensor(out=mu2_st[:], in0=mu_st[:], in1=mu_st[:], op=ALU.mult)
                var_st = wk.tile([128, 4], f32, tag="var_st")
                nc.vector.tensor_tensor(out=var_st[:], in0=sq_st[:, gsl], in1=mu2_st[:], op=ALU.subtract)
                nc.vector.tensor_scalar_add(var_st[:], var_st[:], float(LN_EPS))
                sd_st = wk.tile([128, 4], f32, tag="sd_st")
                nc.scalar.activation(sd_st[:], var_st[:], AF.Sqrt)
                al_st = wk.tile([128, 4], f32, tag="al_st")
                nc.vector.reciprocal(al_st[:], sd_st[:])
                nc.vector.tensor_scalar_mul(gal_st[:, gsl], al_st[:], gh)
                nc.vector.scalar_tensor_tensor(out=ngm_st[:, gsl], in0=mu_st[:], scalar=-1.0,
                                               in1=gal_st[:, gsl], op0=ALU.mult, op1=ALU.mult)
                for k, tt in enumerate(range(4 * G, 4 * G + 4)):
                    nc.scalar.activation(ymg[tt][:], ym32_g[k][:], AF.Identity,
                                         scale=gal_st[:, tt:tt + 1],
                                         bias=ngm_st[:, tt:tt + 1])

                # ---- y accumulation + one (lagged) delta chunk per t-tile
                for I in range(4 * G, 4 * G + 4):
                    y_ps = yps.tile([128, D], f32, tag="yps", name="y_ps")
                    off = I * 128 - t0
                    for J in range(I + 1):
                        nc.tensor.matmul(y_ps[:], sT[J][:, off:off + 128], v_sb[J][:],
                                         start=(J == 0), stop=(J == I))
                    y_out = outp.tile([128, D], f32, tag="y_out")
                    nc.vector.tensor_tensor(out=y_out[:], in0=y_ps[:],
                                            in1=ymg[I][:], op=ALU.add)
                    nc.sync.dma_start(y_d[h, I], y_out[:])

                    cc = I
                    base = max(cc - 2, 0)
                    at_sb = atp.tile([128, 128], bf, tag="at")
                    nc.sync.dma_start(at_sb[:], at_d[h, cc])
                    r_ps = mmps.tile([128, D], f32, tag="mm", name="r_ps")
                    mb = msb[base]
                    has_g1 = cc - 1 >= base
                    has_g2 = cc - 2 >= base
                    nc.tensor.matmul(r_ps[:], kmt_sb[0][:, cc * 128:(cc + 1) * 128],
                                     mb[0][:], start=True, stop=False)
                    nc.tensor.matmul(r_ps[:], kmt_sb[1][:, cc * 128:(cc + 1) * 128],
                                     mb[1][:], start=False,
                                     stop=not (has_g1 or has_g2))
                    if has_g2:
                        nc.tensor.matmul(r_ps[:], g2t_sb[cc][:], usb[cc - 2][:],
                                         start=False, stop=not has_g1)
                    if has_g1:
                        nc.tensor.matmul(r_ps[:], g1t_sb[cc][:], usb[cc - 1][:],
                                         start=False, stop=True)
                    u_in = wk.tile([128, D], bf, tag="u_in")
                    nc.vector.tensor_tensor(out=u_in[:], in0=xn_sb[cc][:],
                                            in1=r_ps[:], op=ALU.subtract)
                    u_ps = mmps.tile([128, D], f32, tag="mm", name="u_ps")
                    nc.tensor.matmul(u_ps[:], at_sb[:], u_in[:], start=True, stop=True)
                    u_sb = dl.tile([128, D], bf, tag="u_sb", name="u_sb")
                    nc.vector.tensor_copy(u_sb[:], u_ps[:])
                    usb[cc] = u_sb
                    last = (cc == NCHUNK - 1)
                    for i in range(2):
                        nc.tensor.matmul(m_ps[i][:],
                                         kmn_sb[cc][:, i * 128:(i + 1) * 128],
                                         u_sb[:], start=False, stop=last)
                    # lagged state copy (used 2 chunks later; off critical path)
                    if cc + 1 <= NCHUNK - 3:
                        nmsb = [dl.tile([128, D], bf, tag=f"msb{i}", name=f"msb{i}")
                                for i in range(2)]
                        nc.scalar.activation(nmsb[0][:], m_ps[0][:], AF.Copy)
                        nc.vector.tensor_copy(nmsb[1][:], m_ps[1][:])
                        msb[cc + 1] = nmsb

            # ---- M_new evacuation
            for i in range(2):
                mn_sb = outp.tile([128, D], f32, tag="mn")
                nc.vector.tensor_copy(mn_sb[:], m_ps[i][:])
                nc.sync.dma_start(mnew_d[h, i], mn_sb[:])

    nc.compile()
    return nc


def _get_program():
    global _PROGRAM
    if _PROGRAM is None:
        _PROGRAM = build_program()
    return _PROGRAM


# --------------------------------------------------------------------------
# public entry
# --------------------------------------------------------------------------

def _run(inputs, trace=False):
    nc = _get_program()
    in_maps = host_prep(inputs)
    kw = dict(trace=True, trace_cores=[0]) if trace else {}
    res = run_bass_kernel_spmd(nc, in_maps, list(range(NCORES)), **kw)
    y = np.zeros((B, NH, T, D), np.float32)
    M_new = np.zeros((B, NH, N, D), np.float32)
    for c in range(NCORES):
        b = c // 2
        h0 = (c % 2) * HPC
        y[b, h0:h0 + HPC] = res.results[c]["y"].reshape(HPC, T, D)
        M_new[b, h0:h0 + HPC] = res.results[c]["mnew"].reshape(HPC, N, D)
    return y, M_new, res.exec_time_ns


def kernel(**inputs):
    y, M_new, _ = _run(inputs, trace=False)
    return y, M_new


def run_profiled(inputs):
    return _run(inputs, trace=True)


# revision 8
# speedup vs baseline: 1.6438x; 1.0751x over previous
"""Trainium2 Bass kernel for nn_Attention_4612794875918.

Full inputs in, full outputs out. Internally shards across 8 NeuronCores:
core c handles batch b = c//2, head group hg = c%2 (4 heads each) — scores,
y_standard, memory matrix M and the delta scan are independent per (B, nh).

Device does all O(T^2) / O(T*N*D) matmul work per (b,h):
  - scoresT = rope(Q) @ rope(Q)^T (strictly-causal, computed transposed,
    staged bf16 in SBUF), y_std = scoresT^T @ V accumulated in PSUM
  - y_mem = Qm @ M0 with per-row LayerNorm fused via ACT scale/bias
  - delta-rule scan in 16 chunks of 128 with a 2-chunk-lagged state copy:
    R_c = K_c M_(c-2) + G2_c U_(c-2) + G1_c U_(c-1);  U_c = A_c (V_c - R_c);
    M += K_c^T U_c, M resident in PSUM fp32 across the scan. The lag keeps
    the PSUM->SBUF state copy off the PE critical path.
Host prep (cheap, O(T*N) / O(T*D)): RoPE tables + rotation of Q, the two
l2-normalized projections, beta/sigmoid, per-chunk triangular solve operators
A = (I + diag(b) tril(K K^T,-1))^{-1} diag(b), the chunk-coupling Gram
matrices G1/G2, layout transposes, bf16 casts.
"""
import math
import sys

import numpy as np
from ml_dtypes import bfloat16

if "/opt/trn_rl_repo" not in sys.path:
    sys.path.insert(0, "/opt/trn_rl_repo")

from contextlib import ExitStack

from concourse import bacc, mybir, tile  # noqa: E402
from concourse.bass_utils import run_bass_kernel_spmd  # noqa: E402

dt = mybir.dt
AF = mybir.ActivationFunctionType
ALU = mybir.AluOpType
AXL = mybir.AxisListType

B, NH, T, N, D = 4, 8, 2048, 256, 512
THETA = 2 ** 16
TWO_PI = 2.0 * math.pi
LN_EPS = 1e-5
C = 128              # delta chunk
NCHUNK = T // C      # 16
NT = T // 128        # 16 t-tiles
HPC = 4              # heads per core
NCORES = 8

_PROGRAM = None      # compile once per process


# --------------------------------------------------------------------------
# host prep
# --------------------------------------------------------------------------

def host_prep(inputs):
    Q = np.asarray(inputs["Q"], np.float32)
    V = np.asarray(inputs["V"], np.float32)
    x_raw = np.asarray(inputs["x_raw"], np.float32)
    x_next = np.asarray(inputs["x_next"], np.float32)
    Wq = np.asarray(inputs["theta_Q_w"], np.float32)
    Wk = np.asarray(inputs["theta_K_w"], np.float32)
    bw = np.asarray(inputs["beta_w"], np.float32)
    mg = np.asarray(inputs["memory_gate"], np.float32)
    M0 = np.asarray(inputs["M0"], np.float32)

    # rope -> QRT bf16 [B,NH,N,T]
    i = np.arange(N, dtype=np.float32)
    q = np.floor(i / 2.0) * 2.0
    freqs = (1.0 / (THETA ** (q / N)) / TWO_PI)
    ph = np.mod(np.arange(T, dtype=np.float32)[:, None] * freqs[None, :], 1.0) * TWO_PI
    pc, ps = np.cos(ph).astype(np.float32), np.sin(ph).astype(np.float32)
    Qe, Qo = Q[..., ::2], Q[..., 1::2]
    Qrot = np.empty_like(Q)
    Qrot[..., ::2] = -Qo
    Qrot[..., 1::2] = Qe
    QR = Q * pc + Qrot * ps
    QRT = np.ascontiguousarray(np.swapaxes(QR, -1, -2)).astype(bfloat16)

    Qm = x_raw @ Wq.T
    Qm /= np.maximum(np.linalg.norm(Qm, axis=-1, keepdims=True), 1e-12)
    Km = x_raw @ Wk.T
    Km /= np.maximum(np.linalg.norm(Km, axis=-1, keepdims=True), 1e-12)
    QmT = np.ascontiguousarray(np.swapaxes(Qm, -1, -2)).astype(bfloat16)  # [B,N,T]
    KmT = np.ascontiguousarray(np.swapaxes(Km, -1, -2)).astype(bfloat16)
    Kmn = Km.astype(bfloat16)                                            # [B,T,N]

    beta = 1.0 / (1.0 + np.exp(-(x_raw @ bw.T)))                         # [B,T,NH]
    KmC = Km.reshape(B, NCHUNK, C, N)
    S = np.einsum("bcik,bcjk->bcij", KmC, KmC)
    S_L = np.tril(S, -1)
    Ieye = np.eye(C, dtype=np.float32)
    bC = beta.reshape(B, NCHUNK, C, NH).transpose(0, 3, 1, 2)            # [B,NH,NCHUNK,C]
    Mats = Ieye[None, None, None] + bC[..., None] * S_L[:, None]
    A = np.linalg.inv(Mats) * bC[:, :, :, None, :]                       # [B,NH,NCHUNK,C,C]
    AT = np.ascontiguousarray(np.swapaxes(A, -1, -2)).astype(bfloat16)

    # chunk-coupling Gram matrices in lhsT form:
    # g1t[c] = Km_(c-1) Km_c^T (c>=1), g2t[c] = Km_(c-2) Km_c^T (c>=2)
    g1t = np.zeros((B, NCHUNK, C, C), np.float32)
    g2t = np.zeros((B, NCHUNK, C, C), np.float32)
    for b in range(B):
        for c in range(1, NCHUNK):
            g1t[b, c] = KmC[b, c - 1] @ KmC[b, c].T
        for c in range(2, NCHUNK):
            g2t[b, c] = KmC[b, c - 2] @ KmC[b, c].T
    g1t = g1t.astype(bfloat16)
    g2t = g2t.astype(bfloat16)

    g = (1.0 / (1.0 + np.exp(-mg.reshape(NH)))).astype(np.float32)

    masku = np.triu(np.ones((128, 128), np.float32), 1)
    ident = np.eye(128, dtype=np.float32).astype(bfloat16)

    in_maps = []
    for c in range(NCORES):
        b = c // 2
        h0 = (c % 2) * HPC
        gloc = g[h0:h0 + HPC]
        def pmajor(x, tiles, width, dtype):
            # [tiles,128,width] -> [128, tiles*width] partition-major
            return np.ascontiguousarray(
                x.reshape(tiles, 128, width).transpose(1, 0, 2).reshape(128, tiles * width)
            ).astype(dtype)

        in_maps.append(dict(
            qrt=np.ascontiguousarray(QRT[b, h0:h0 + HPC]).reshape(HPC, 2, 128, T),
            qmt=np.ascontiguousarray(QmT[b]).reshape(2, 128, T),
            kmt=np.ascontiguousarray(KmT[b]).reshape(2, 128, T),
            kmn=pmajor(Kmn[b], NT, N, bfloat16),
            at=np.stack([pmajor(AT[b, h0 + j], NCHUNK, 128, bfloat16) for j in range(HPC)]),
            g1t=pmajor(g1t[b], NCHUNK, 128, bfloat16),
            g2t=pmajor(g2t[b], NCHUNK, 128, bfloat16),
            v=np.stack([pmajor(V[b, h0 + j], NT, D, bfloat16) for j in range(HPC)]),
            xn=pmajor(x_next[b], NT, D, np.float32),
            m0=np.stack([pmajor(M0[b, h0 + j], 2, D, bfloat16) for j in range(HPC)]),
            masku=masku,
            ident=ident,
            gcol=np.broadcast_to(gloc, (128, HPC)).copy(),
            g1col=np.broadcast_to(1.0 - gloc, (128, HPC)).copy(),
        ))
    return in_maps


# --------------------------------------------------------------------------
# device program (identical on all cores)
# --------------------------------------------------------------------------

def build_program():
    nc = bacc.Bacc("TRN2", target_bir_lowering=False, debug=False,
                   num_devices=NCORES)
    bf = dt.bfloat16
    f32 = dt.float32

    qrt_d = nc.dram_tensor("qrt", [HPC, 2, 128, T], bf, kind="ExternalInput").ap()
    qmt_d = nc.dram_tensor("qmt", [2, 128, T], bf, kind="ExternalInput").ap()
    kmt_d = nc.dram_tensor("kmt", [2, 128, T], bf, kind="ExternalInput").ap()
    kmn_d = nc.dram_tensor("kmn", [128, NT * N], bf, kind="ExternalInput").ap()
    at_d = nc.dram_tensor("at", [HPC, 128, NCHUNK * 128], bf, kind="ExternalInput").ap()
    g1t_d = nc.dram_tensor("g1t", [128, NCHUNK * 128], bf, kind="ExternalInput").ap()
    g2t_d = nc.dram_tensor("g2t", [128, NCHUNK * 128], bf, kind="ExternalInput").ap()
    v_d = nc.dram_tensor("v", [HPC, 128, NT * D], bf, kind="ExternalInput").ap()
    xn_d = nc.dram_tensor("xn", [128, NT * D], f32, kind="ExternalInput").ap()
    m0_d = nc.dram_tensor("m0", [HPC, 128, 2 * D], bf, kind="ExternalInput").ap()
    masku_d = nc.dram_tensor("masku", [128, 128], f32, kind="ExternalInput").ap()
    ident_d = nc.dram_tensor("ident", [128, 128], bf, kind="ExternalInput").ap()
    gcol_d = nc.dram_tensor("gcol", [128, HPC], f32, kind="ExternalInput").ap()
    g1col_d = nc.dram_tensor("g1col", [128, HPC], f32, kind="ExternalInput").ap()
    y_d = nc.dram_tensor("y", [HPC, NT, 128, D], f32, kind="ExternalOutput").ap()
    mnew_d = nc.dram_tensor("mnew", [HPC, 2, 128, D], f32, kind="ExternalOutput").ap()

    with tile.TileContext(nc) as tc, ExitStack() as ctx:
        cst = ctx.enter_context(tc.tile_pool(name="cst", bufs=1))
        qrtp = ctx.enter_context(tc.tile_pool(name="qrtp", bufs=2))
        vp = ctx.enter_context(tc.tile_pool(name="vp", bufs=2))
        stp = ctx.enter_context(tc.tile_pool(name="stp", bufs=18))
        ymgp = ctx.enter_context(tc.tile_pool(name="ymgp", bufs=6))
        wk = ctx.enter_context(tc.tile_pool(name="wk", bufs=2))
        dl = ctx.enter_context(tc.tile_pool(name="dl", bufs=4))      # delta lagged state
        m0p = ctx.enter_context(tc.tile_pool(name="m0p", bufs=2))
        atp = ctx.enter_context(tc.tile_pool(name="atp", bufs=2))
        outp = ctx.enter_context(tc.tile_pool(name="outp", bufs=3))
        mmps = ctx.enter_context(tc.tile_pool(name="mmps", bufs=4, space="PSUM"))
        yps = ctx.enter_context(tc.tile_pool(name="yps", bufs=2, space="PSUM"))
        mps = ctx.enter_context(tc.tile_pool(name="mps", bufs=2, space="PSUM"))

        # ---- persistent constants / per-core tensors
        masku_sb = cst.tile([128, 128], f32, tag="masku")
        ident_sb = cst.tile([128, 128], bf, tag="ident")
        gcol_sb = cst.tile([128, HPC], f32, tag="gcol")
        g1col_sb = cst.tile([128, HPC], f32, tag="g1col")
        zero_sb = cst.tile([128, 1], f32, tag="zero")
        nc.sync.dma_start(masku_sb[:], masku_d[:])
        nc.sync.dma_start(ident_sb[:], ident_d[:])
        nc.sync.dma_start(gcol_sb[:], gcol_d[:])
        nc.sync.dma_start(g1col_sb[:], g1col_d[:])
        nc.gpsimd.memset(zero_sb[:], 0.0)

        qmt_sb = [cst.tile([128, T], bf, tag=f"qmt{i}", name=f"qmt_sb{i}") for i in range(2)]
        kmt_sb = [cst.tile([128, T], bf, tag=f"kmt{i}", name=f"kmt_sb{i}") for i in range(2)]
        for i in range(2):
            nc.sync.dma_start(qmt_sb[i][:], qmt_d[i])
            nc.sync.dma_start(kmt_sb[i][:], kmt_d[i])
        kmn_sb = cst.tile([128, NT * N], bf, tag="kmn")
        xn_sb = cst.tile([128, NT * D], f32, tag="xn")
        g1t_sb = cst.tile([128, NCHUNK * 128], bf, tag="g1t")
        g2t_sb = cst.tile([128, NCHUNK * 128], bf, tag="g2t")

        for h in range(HPC):
            gh = gcol_sb[:, h:h + 1]
            g1h = g1col_sb[:, h:h + 1]

            qrt_sb = [qrtp.tile([128, T], bf, tag="qrt", name="qrt_sb") for _ in range(2)]
            for i in range(2):
                nc.sync.dma_start(qrt_sb[i][:], qrt_d[h, i])
            v_sb = vp.tile([128, NT * D], bf, tag="v")
            nc.sync.dma_start(v_sb[:], v_d[h])
            m0b_big = m0p.tile([128, 2 * D], bf, tag="m0b")
            nc.sync.dma_start(m0b_big[:], m0_d[h])
            m0b_sb = [m0b_big[:, i * D:(i + 1) * D] for i in range(2)]
            at_big = atp.tile([128, NCHUNK * 128], bf, tag="at")
            nc.sync.dma_start(at_big[:], at_d[h])
            if h == 0:
                # heavy shared loads deferred behind pair-0 working set
                nc.sync.dma_start(kmn_sb[:], kmn_d[:])
                nc.sync.dma_start(xn_sb[:], xn_d[:])
                nc.sync.dma_start(g1t_sb[:], g1t_d[:])
                nc.sync.dma_start(g2t_sb[:], g2t_d[:])

            # delta state: M in PSUM fp32, lagged bf16 copies + U history in SBUF
            m_ps = [mps.tile([128, D], f32, tag="mps", name="m_ps") for _ in range(2)]
            for i in range(2):
                nc.tensor.matmul(m_ps[i][:], ident_sb[:], m0b_sb[i][:],
                                 start=True, stop=False)
            msb = {0: m0b_sb}     # msb[c] = bf16 state before chunk c (lagged)
            usb = {}              # usb[c] = bf16 U_c

            ym_done = False
            sum_st = wk.tile([128, NT], f32, tag="sum_st")
            sq_st = wk.tile([128, NT], f32, tag="sq_st")
            gal_st = wk.tile([128, NT], f32, tag="gal_st")
            ngm_st = wk.tile([128, NT], f32, tag="ngm_st")
            ymg = [ymgp.tile([128, D], bf, tag="ymg", name="ymg_sb") for _ in range(NT)]

            for G in range(4):
                t0 = G * 512
                # ---- scoresT generation for this G window
                sT = {}
                for J in range(4 * G + 4):
                    sc_ps = mmps.tile([128, 512], f32, tag="mm", name="sc_ps")
                    nc.tensor.matmul(sc_ps[:], qrt_sb[0][:, J * 128:(J + 1) * 128],
                                     qrt_sb[0][:, t0:t0 + 512], start=True, stop=False)
                    nc.tensor.matmul(sc_ps[:], qrt_sb[1][:, J * 128:(J + 1) * 128],
                                     qrt_sb[1][:, t0:t0 + 512], start=False, stop=True)
                    st_sb = stp.tile([128, 512], bf, tag="st", name="st_sb")
                    off = J * 128 - t0
                    if off >= 0:
                        if off > 0:
                            nc.gpsimd.memset(st_sb[:, 0:off], 0.0)
                        nc.vector.scalar_tensor_tensor(
                            out=st_sb[:, off:off + 128], in0=sc_ps[:, off:off + 128],
                            scalar=g1h, in1=masku_sb[:], op0=ALU.mult, op1=ALU.mult)
                        if off + 128 < 512:
                            nc.vector.tensor_scalar_mul(
                                st_sb[:, off + 128:512], sc_ps[:, off + 128:512], g1h)
                    else:
                        if J % 2 == 0:
                            nc.vector.tensor_scalar_mul(st_sb[:], sc_ps[:], g1h)
                        else:
                            nc.scalar.activation(st_sb[:], sc_ps[:], AF.Identity,
                                                 scale=g1h, bias=zero_sb[:])
                    sT[J] = st_sb

                # ---- y_memory for this G's 4 t-tiles (spread across groups)
                ym32_g = []
                for tt in range(4 * G, 4 * G + 4):
                    ym_ps = mmps.tile([128, D], f32, tag="mm", name="ym_ps")
                    nc.tensor.matmul(ym_ps[:], qmt_sb[0][:, tt * 128:(tt + 1) * 128],
                                     m0b_sb[0][:], start=True, stop=False)
                    nc.tensor.matmul(ym_ps[:], qmt_sb[1][:, tt * 128:(tt + 1) * 128],
                                     m0b_sb[1][:], start=False, stop=True)
                    scr = wk.tile([128, D], bf, tag="scr")
                    nc.scalar.activation(scr[:], ym_ps[:], AF.Square,
                                         scale=float(1.0 / math.sqrt(D)),
                                         accum_out=sq_st[:, tt:tt + 1])
                    ym32 = ymgp.tile([128, D], bf, tag="ym32", name="ym32_sb")
                    nc.scalar.activation(ym32[:], ym_ps[:], AF.Copy,
                                         accum_out=sum_st[:, tt:tt + 1])
                    ym32_g.append(ym32)
                gsl = slice(4 * G, 4 * G + 4)
                mu_st = wk.tile([128, 4], f32, tag="mu_st")
                nc.vector.tensor_scalar_mul(mu_st[:], sum_st[:, gsl], float(1.0 / D))
                mu2_st = wk.tile([128, 4], f32, tag="mu2_st")
                nc.vector.tensor_tensor(out=mu2_st[:], in0=mu_st[:], in1=mu_st[:], op=ALU.mult)
                var_st = wk.tile([128, 4], f32, tag="var_st")
                nc.vector.tensor_tensor(out=var_st[:], in0=sq_st[:, gsl], in1=mu2_st[:], op=ALU.subtract)
                nc.vector.tensor_scalar_add(var_st[:], var_st[:], float(LN_EPS))
                sd_st = wk.tile([128, 4], f32, tag="sd_st")
                nc.scalar.activation(sd_st[:], var_st[:], AF.Sqrt)
                al_st = wk.tile([128, 4], f32, tag="al_st")
                nc.vector.reciprocal(al_st[:], sd_st[:])
                nc.vector.tensor_scalar_mul(gal_st[:, gsl], al_st[:], gh)
                nc.vector.scalar_tensor_tensor(out=ngm_st[:, gsl], in0=mu_st[:], scalar=-1.0,
                                               in1=gal_st[:, gsl], op0=ALU.mult, op1=ALU.mult)
                for k, tt in enumerate(range(4 * G, 4 * G + 4)):
                    nc.scalar.activation(ymg[tt][:], ym32_g[k][:], AF.Identity,
                                         scale=gal_st[:, tt:tt + 1],
                                         bias=ngm_st[:, tt:tt + 1])

                # ---- y accumulation + one (lagged) delta chunk per t-tile
                for I in range(4 * G, 4 * G + 4):
                    y_ps = yps.tile([128, D], f32, tag="yps", name="y_ps")
                    off = I * 128 - t0
                    for J in range(I + 1):
                        nc.tensor.matmul(y_ps[:], sT[J][:, off:off + 128], v_sb[:, J * D:(J + 1) * D],
                                         start=(J == 0), stop=(J == I))
                    y_out = outp.tile([128, D], f32, tag="y_out")
                    nc.vector.tensor_tensor(out=y_out[:], in0=y_ps[:],
                                            in1=ymg[I][:], op=ALU.add)
                    nc.sync.dma_start(y_d[h, I], y_out[:])

                    cc = I
                    base = max(cc - 2, 0)
                    r_ps = mmps.tile([128, D], f32, tag="mm", name="r_ps")
                    mb = msb[base]
                    has_g1 = cc - 1 >= base
                    has_g2 = cc - 2 >= base
                    nc.tensor.matmul(r_ps[:], kmt_sb[0][:, cc * 128:(cc + 1) * 128],
                                     mb[0][:], start=True, stop=False)
                    nc.tensor.matmul(r_ps[:], kmt_sb[1][:, cc * 128:(cc + 1) * 128],
                                     mb[1][:], start=False,
                                     stop=not (has_g1 or has_g2))
                    if has_g2:
                        nc.tensor.matmul(r_ps[:], g2t_sb[:, cc * 128:(cc + 1) * 128], usb[cc - 2][:],
                                         start=False, stop=not has_g1)
                    if has_g1:
                        nc.tensor.matmul(r_ps[:], g1t_sb[:, cc * 128:(cc + 1) * 128], usb[cc - 1][:],
                                         start=False, stop=True)
                    u_in = wk.tile([128, D], bf, tag="u_in")
                    nc.vector.tensor_tensor(out=u_in[:], in0=xn_sb[:, cc * D:(cc + 1) * D],
                                            in1=r_ps[:], op=ALU.subtract)
                    u_ps = mmps.tile([128, D], f32, tag="mm", name="u_ps")
                    nc.tensor.matmul(u_ps[:], at_big[:, cc * 128:(cc + 1) * 128],
                                     u_in[:], start=True, stop=True)
                    u_sb = dl.tile([128, D], bf, tag="u_sb", name="u_sb")
                    nc.vector.tensor_copy(u_sb[:], u_ps[:])
                    usb[cc] = u_sb
                    last = (cc == NCHUNK - 1)
                    for i in range(2):
                        nc.tensor.matmul(m_ps[i][:],
                                         kmn_sb[:, cc * N + i * 128: cc * N + (i + 1) * 128],
                                         u_sb[:], start=False, stop=last)
                    # lagged state copy (used 2 chunks later; off critical path)
                    if cc + 1 <= NCHUNK - 3:
                        nmsb = [dl.tile([128, D], bf, tag=f"msb{i}", name=f"msb{i}")
                                for i in range(2)]
                        nc.scalar.activation(nmsb[0][:], m_ps[0][:], AF.Copy)
                        nc.vector.tensor_copy(nmsb[1][:], m_ps[1][:])
                        msb[cc + 1] = nmsb

            # ---- M_new evacuation
            for i in range(2):
                mn_sb = outp.tile([128, D], f32, tag="mn")
                nc.vector.tensor_copy(mn_sb[:], m_ps[i][:])
                nc.sync.dma_start(mnew_d[h, i], mn_sb[:])

    nc.compile()
    return nc


def _get_program():
    global _PROGRAM
    if _PROGRAM is None:
        _PROGRAM = build_program()
    return _PROGRAM


# --------------------------------------------------------------------------
# public entry
# --------------------------------------------------------------------------

def _run(inputs, trace=False):
    nc = _get_program()
    in_maps = host_prep(inputs)
    kw = dict(trace=True, trace_cores=[0]) if trace else {}
    res = run_bass_kernel_spmd(nc, in_maps, list(range(NCORES)), **kw)
    y = np.zeros((B, NH, T, D), np.float32)
    M_new = np.zeros((B, NH, N, D), np.float32)
    for c in range(NCORES):
        b = c // 2
        h0 = (c % 2) * HPC
        y[b, h0:h0 + HPC] = res.results[c]["y"].reshape(HPC, T, D)
        M_new[b, h0:h0 + HPC] = res.results[c]["mnew"].reshape(HPC, N, D)
    return y, M_new, res.exec_time_ns


def kernel(**inputs):
    y, M_new, _ = _run(inputs, trace=False)
    return y, M_new


def run_profiled(inputs):
    return _run(inputs, trace=True)
